# revision 7
# baseline (speedup 1.0000x reference)
"""CIF (continuous integrate-and-fire) Trainium2 Bass kernel.

Math: after scaling alpha so it sums to target_len, the CIF scan is a
segment-reduce: token k = sum_t W[k,t] * h[t] with
W[k,t] = |[k,k+1) ∩ [C_{t-1}, C_t)|  (C = cumsum of scaled alpha), i.e.
W[t->row, k->col] = clip01(C_t - k) - clip01(C_{t-1} - k).
Fire frame of token k: t_k = #{t : C_t < k+1}; boundaries[k] = [t_{k-1}, t_k+1],
tail token gets [t_{F-1}, T].  All computed on-device; batch is sharded
2 sequences per core across 8 cores.

Precision: C is computed two-level (per-125-frame scan + chunk-offset prefix),
with chunk totals split into an exact 2^-10-grid part and a tiny residual so
the offset prefix sums are exact; thresholds are evaluated as
(offset_hi - k) + offset_lo + C_local, keeping comparisons at ~1e-5 accuracy.
"""

import numpy as np
from contextlib import ExitStack

import concourse.bacc as bacc
import concourse.bass as bass
import concourse.tile as tile
from concourse import mybir
from concourse.bass_utils import run_bass_kernel_spmd

B, T, D = 16, 2000, 768
NCORES = 8
SPC = 2              # sequences per core
CHUNK = 125
NCHUNK = 16          # 2000 = 16 * 125
NTOK = 256           # max token index is 255 (target_len < 256)
NMAX = 384
MAGIC = 12582912.0   # 1.5 * 2^23: fp32 round-to-int via add/sub

F32 = mybir.dt.float32
F32R = mybir.dt.float32r
F16 = mybir.dt.float16
I32 = mybir.dt.int32
BF16 = mybir.dt.bfloat16
OP = mybir.AluOpType
AF = mybir.ActivationFunctionType


def _consts():
    lt32 = np.zeros((32, 32), np.float32)   # lt32[q,p]=1 iff same 16-block, q%16<p%16
    for q in range(32):
        for p in range(32):
            if q // 16 == p // 16 and q % 16 < p % 16:
                lt32[q, p] = 1.0
    sum32 = np.zeros((32, 2), np.float32)   # sum32[q,s]=1 iff q//16==s
    for q in range(32):
        sum32[q, q // 16] = 1.0
    id32 = np.eye(32, dtype=np.float32)
    return lt32, sum32, id32


def _bcast_ap(ap, n):
    """Partition-broadcast source AP for DMA: read the same row n times.

    `ap` must be a [1, m] (or [1]-leading) access pattern; the leading
    singleton is replaced by a zero-step dim of count n."""
    return bass.AP(tensor=ap.tensor, offset=ap.offset,
                   ap=[[0, n]] + [list(d) for d in ap.ap[1:]])


def _body(ctx, tc, outs, ins):
    nc = tc.nc
    c_out, tl_out, bd_out = outs
    h_in, a_in, tl_in = ins

    const = ctx.enter_context(tc.tile_pool(name="const", bufs=1))
    setup = ctx.enter_context(tc.tile_pool(name="setup", bufs=1))
    hpool = ctx.enter_context(tc.tile_pool(name="hp", bufs=4))
    work = ctx.enter_context(tc.tile_pool(name="work", bufs=3))
    drain = ctx.enter_context(tc.tile_pool(name="drain", bufs=2))
    fin = ctx.enter_context(tc.tile_pool(name="fin", bufs=2))
    ps_acc = ctx.enter_context(tc.tile_pool(name="psacc", bufs=1, space="PSUM"))
    ps_sm = ctx.enter_context(tc.tile_pool(name="pssm", bufs=2, space="PSUM"))

    # ---- static tiles (constants embedded in the NEFF) ----
    lt32_np, sum32_np, id32_np = _consts()
    lt32 = const.tile([32, 32], F32)
    nc.sync.dma_start(lt32[:], nc.inline_tensor(lt32_np, "lt32_c").ap())
    sum32 = const.tile([32, 2], F32)
    nc.sync.dma_start(sum32[:], nc.inline_tensor(sum32_np, "sum32_c").ap())
    id32 = const.tile([32, 32], F32)
    nc.sync.dma_start(id32[:], nc.inline_tensor(id32_np, "id32_c").ap())

    negio_i = const.tile([CHUNK, NTOK], I32)
    nc.gpsimd.iota(negio_i[:], pattern=[[1, NTOK]], base=0, channel_multiplier=0)
    negio = const.tile([CHUNK, NTOK], F32)
    nc.vector.tensor_scalar(out=negio[:], in0=negio_i[:], scalar1=-1.0,
                            scalar2=None, op0=OP.mult)
    kidx_i = const.tile([1, NTOK], I32)
    nc.gpsimd.iota(kidx_i[:], pattern=[[1, NTOK]], base=0, channel_multiplier=0)
    kidx = const.tile([1, NTOK], F32)
    nc.vector.tensor_copy(out=kidx[:], in_=kidx_i[:])
    ones_b = const.tile([CHUNK, 1], BF16)
    nc.vector.memset(ones_b[:], 1.0)

    # ---- per-core setup: scale, scan, offsets ----
    al = setup.tile([32, CHUNK], F32)
    nc.sync.dma_start(al[:], a_in.rearrange("s (c f) -> (s c) f", f=CHUNK))
    tl_i = setup.tile([SPC, 1], I32)
    nc.sync.dma_start(tl_i[:], tl_in.rearrange("(s one) -> s one", one=1))
    tlf = setup.tile([SPC, 1], F32)
    nc.vector.tensor_copy(out=tlf[:], in_=tl_i[:])

    tot_raw = setup.tile([32, 1], F32)
    nc.vector.reduce_sum(out=tot_raw[:], in_=al[:], axis=mybir.AxisListType.X)
    ps_fsr = ps_sm.tile([SPC, 1], F32, tag="sm")
    nc.tensor.matmul(ps_fsr[:], sum32[:], tot_raw[:], start=True, stop=True,
                     skip_group_check=True)
    # scale = tl / (sum + 1e-6), reciprocal + 1 Newton step
    dn = setup.tile([SPC, 1], F32)
    nc.vector.tensor_scalar(out=dn[:], in0=ps_fsr[:], scalar1=1e-6, scalar2=None,
                            op0=OP.add)
    r0 = setup.tile([SPC, 1], F32)
    nc.vector.reciprocal(out=r0[:], in_=dn[:])
    t0 = setup.tile([SPC, 1], F32)
    nc.vector.tensor_tensor(out=t0[:], in0=dn[:], in1=r0[:], op=OP.mult)
    u0 = setup.tile([SPC, 1], F32)
    nc.vector.tensor_scalar(out=u0[:], in0=t0[:], scalar1=-1.0, scalar2=2.0,
                            op0=OP.mult, op1=OP.add)
    r1n = setup.tile([SPC, 1], F32)
    nc.vector.tensor_tensor(out=r1n[:], in0=r0[:], in1=u0[:], op=OP.mult)
    scl = setup.tile([SPC, 1], F32)
    nc.vector.tensor_tensor(out=scl[:], in0=tlf[:], in1=r1n[:], op=OP.mult)
    sbc = setup.tile([32, 1], F32)
    scr_scl = nc.dram_tensor("scr_scl", [SPC, 1], F32, kind="Internal").ap()
    nc.sync.dma_start(scr_scl[:], scl[:])
    for s in range(SPC):
        row = scr_scl[s:s + 1, :]
        nc.gpsimd.dma_start(sbc[16 * s:16 * (s + 1), :], _bcast_ap(row, 16))

    als = setup.tile([32, CHUNK], F32)
    nc.vector.tensor_scalar(out=als[:], in0=al[:], scalar1=sbc[:], scalar2=None,
                            op0=OP.mult)
    cl = setup.tile([32, CHUNK], F32)
    nc.vector.tensor_tensor_scan(out=cl[:], data0=als[:], data1=als[:],
                                 initial=0.0, op0=OP.add, op1=OP.bypass)
    # exclusive chunk-offset prefix + per-seq final C
    tot = cl[:, CHUNK - 1:CHUNK]
    ps_off = ps_sm.tile([32, 1], F32, tag="sm")
    nc.tensor.matmul(ps_off[:], lt32[:], tot, start=True, stop=True,
                     skip_group_check=True)
    ps_fs = ps_sm.tile([SPC, 1], F32, tag="sm")
    nc.tensor.matmul(ps_fs[:], sum32[:], tot, start=True, stop=True,
                     skip_group_check=True)
    cfin2 = setup.tile([SPC, 1], F32)
    nc.vector.tensor_copy(out=cfin2[:], in_=ps_fs[:])
    ps_cfT = ps_sm.tile([1, SPC], F32, tag="sm")
    nc.tensor.transpose(ps_cfT[:], cfin2[:], id32[0:SPC, 0:SPC])
    cfinT = setup.tile([1, SPC], F32)
    nc.vector.tensor_copy(out=cfinT[:], in_=ps_cfT[:])

    off_sb = setup.tile([32, 1], F32)
    nc.vector.tensor_copy(out=off_sb[:], in_=ps_off[:])
    ps_o = ps_sm.tile([1, 32], F32, tag="sm")
    nc.tensor.transpose(ps_o[:], off_sb[:], id32[:])
    offT0 = setup.tile([1, 32], F32)
    nc.vector.tensor_copy(out=offT0[:], in_=ps_o[:])
    scr_off = nc.dram_tensor("scr_off", [1, 32], F32, kind="Internal").ap()
    nc.sync.dma_start(scr_off[:], offT0[:])
    offh = setup.tile([CHUNK, 32], F32)
    nc.gpsimd.dma_start(offh[:], _bcast_ap(scr_off[0:1, :], CHUNK))

    # transposed local-cumsum columns: CEcur[f,cidx]=Cl[cidx,f], CEprev shifted
    clp = setup.tile([32, CHUNK], F32)
    nc.vector.memset(clp[:, 0:1], 0.0)
    nc.vector.tensor_copy(out=clp[:, 1:CHUNK], in_=cl[:, 0:CHUNK - 1])
    ps_t1 = ps_sm.tile([CHUNK, 32], F32, tag="aux")
    nc.tensor.transpose(ps_t1[:], cl[:], id32[:])
    cecur = setup.tile([CHUNK, 32], F32)
    nc.vector.tensor_copy(out=cecur[:], in_=ps_t1[:])
    ps_t2 = ps_sm.tile([CHUNK, 32], F32, tag="aux")
    nc.tensor.transpose(ps_t2[:], clp[:], id32[:])
    ceprev = setup.tile([CHUNK, 32], F32)
    nc.vector.tensor_copy(out=ceprev[:], in_=ps_t2[:])
    # fold chunk offset into the ACT bias columns
    bias_a = setup.tile([CHUNK, 32], F32)
    nc.vector.tensor_tensor(out=bias_a[:], in0=cecur[:], in1=offh[:], op=OP.add)
    bias_b = setup.tile([CHUNK, 32], F32)
    nc.vector.tensor_tensor(out=bias_b[:], in0=ceprev[:], in1=offh[:], op=OP.add)

    # ---- main loop ----
    for s in range(SPC):
        counts_ps = ps_sm.tile([1, NTOK], F32, tag="aux")
        acc_a0 = ps_acc.tile([128, 512], F32, tag="a0")
        acc_b0 = ps_acc.tile([128, 256], F32, tag="b0")
        acc_a1 = ps_acc.tile([128, 512], F32, tag="a1")
        acc_b1 = ps_acc.tile([128, 256], F32, tag="b1")
        for c in range(NCHUNK):
            cidx = 16 * s + c
            ht = hpool.tile([CHUNK, D], F32, tag="h")
            nc.sync.dma_start(ht[:], h_in[s, c * CHUNK:(c + 1) * CHUNK, :])
            r1a = work.tile([CHUNK, NTOK], F32, tag="r1a")
            nc.scalar.activation(out=r1a[:], in_=negio[:], func=AF.Relu,
                                 bias=bias_a[:, cidx:cidx + 1], scale=1.0)
            r1b = work.tile([CHUNK, NTOK], F32, tag="r1b")
            nc.scalar.activation(out=r1b[:], in_=negio[:], func=AF.Relu,
                                 bias=bias_b[:, cidx:cidx + 1], scale=1.0)
            pb = work.tile([CHUNK, NTOK], F32, tag="pb")
            nc.vector.tensor_scalar(out=pb[:], in0=r1b[:], scalar1=1.0,
                                    scalar2=None, op0=OP.min)
            wt = work.tile([CHUNK, NTOK], F16, tag="w")
            nc.vector.scalar_tensor_tensor(out=wt[:], in0=r1a[:], scalar=1.0,
                                           in1=pb[:], op0=OP.min,
                                           op1=OP.subtract)
            ind = work.tile([CHUNK, NTOK], BF16, tag="ind")
            nc.vector.tensor_scalar(out=ind[:], in0=r1a[:], scalar1=1.0,
                                    scalar2=None, op0=OP.is_lt)
            hrr = work.tile([CHUNK, D], F16, tag="hr")
            if c % 2 == 0:
                nc.scalar.copy(out=hrr[:], in_=ht[:])
            else:
                nc.vector.tensor_copy(out=hrr[:], in_=ht[:])
            st, sp = (c == 0), (c == NCHUNK - 1)
            nc.tensor.matmul(counts_ps[:], ones_b[:], ind[:], start=st, stop=sp,
                             skip_group_check=True)
            nc.tensor.matmul(acc_a0[:], wt[:, 0:128], hrr[:, 0:512], start=st,
                             stop=sp, skip_group_check=True)
            nc.tensor.matmul(acc_b0[:], wt[:, 0:128], hrr[:, 512:768], start=st,
                             stop=sp, skip_group_check=True)
            nc.tensor.matmul(acc_a1[:], wt[:, 128:256], hrr[:, 0:512], start=st,
                             stop=sp, skip_group_check=True)
            nc.tensor.matmul(acc_b1[:], wt[:, 128:256], hrr[:, 512:768], start=st,
                             stop=sp, skip_group_check=True)

        # ---- finalize sequence s ----
        csb = fin.tile([1, NTOK], F32, tag="csb")
        nc.vector.tensor_copy(out=csb[:], in_=counts_ps[:])
        ltc = fin.tile([1, NTOK], F32, tag="ltc")
        nc.vector.tensor_scalar(out=ltc[:], in0=csb[:], scalar1=float(T),
                                scalar2=None, op0=OP.is_lt)
        ft = fin.tile([1, 1], F32, tag="ft")
        nc.vector.reduce_sum(out=ft[:], in_=ltc[:], axis=mybir.AxisListType.X)
        frac = fin.tile([1, 1], F32, tag="frac")
        nc.vector.tensor_tensor(out=frac[:], in0=cfinT[:, s:s + 1], in1=ft[:],
                                op=OP.subtract)
        tailf = fin.tile([1, 1], F32, tag="tailf")
        nc.vector.tensor_scalar(out=tailf[:], in0=frac[:], scalar1=1e-4,
                                scalar2=None, op0=OP.is_gt)
        ntokf = fin.tile([1, 1], F32, tag="ntokf")
        nc.vector.tensor_tensor(out=ntokf[:], in0=ft[:], in1=tailf[:], op=OP.add)
        tli = fin.tile([1, 1], I32, tag="tli")
        nc.vector.tensor_copy(out=tli[:], in_=ntokf[:])
        nc.sync.dma_start(tl_out[s:s + 1].rearrange("(a b) -> a b", b=1), tli[:])

        valid = fin.tile([1, NTOK], F32, tag="valid")
        nc.vector.tensor_scalar(out=valid[:], in0=kidx[:], scalar1=ntokf[:],
                                scalar2=None, op0=OP.is_lt)
        isfire = fin.tile([1, NTOK], F32, tag="isfire")
        nc.vector.tensor_scalar(out=isfire[:], in0=kidx[:], scalar1=ft[:],
                                scalar2=None, op0=OP.is_lt)
        c1 = fin.tile([1, NTOK], F32, tag="c1")
        nc.vector.tensor_scalar(out=c1[:], in0=csb[:], scalar1=1.0, scalar2=None,
                                op0=OP.add)
        endsA = fin.tile([1, NTOK], F32, tag="endsA")
        nc.vector.tensor_tensor(out=endsA[:], in0=c1[:], in1=isfire[:], op=OP.mult)
        tailm = fin.tile([1, NTOK], F32, tag="tailm")
        nc.vector.tensor_tensor(out=tailm[:], in0=valid[:], in1=isfire[:],
                                op=OP.subtract)
        ends = fin.tile([1, NTOK], F32, tag="ends")
        nc.vector.scalar_tensor_tensor(out=ends[:], in0=tailm[:],
                                       scalar=float(T), in1=endsA[:],
                                       op0=OP.mult, op1=OP.add)
        ssh = fin.tile([1, NTOK], F32, tag="ssh")
        nc.vector.memset(ssh[:, 0:1], 0.0)
        nc.vector.tensor_copy(out=ssh[:, 1:NTOK], in_=csb[:, 0:NTOK - 1])
        starts = fin.tile([1, NTOK], F32, tag="starts")
        nc.vector.tensor_tensor(out=starts[:], in0=ssh[:], in1=valid[:],
                                op=OP.mult)
        ends_i = fin.tile([1, NTOK], I32, tag="ends_i")
        nc.vector.tensor_copy(out=ends_i[:], in_=ends[:])
        starts_i = fin.tile([1, NTOK], I32, tag="starts_i")
        nc.vector.tensor_copy(out=starts_i[:], in_=starts[:])
        nc.sync.dma_start(
            bd_out[s, 0:NTOK, 0].rearrange("(one n) -> one n", one=1), starts_i[:])
        nc.sync.dma_start(
            bd_out[s, 0:NTOK, 1].rearrange("(one n) -> one n", one=1), ends_i[:])

        ct0 = drain.tile([128, D], F32, tag="ct")
        nc.scalar.activation(out=ct0[:, 0:512], in_=acc_a0[:], func=AF.Copy)
        nc.scalar.activation(out=ct0[:, 512:768], in_=acc_b0[:], func=AF.Copy)
        nc.sync.dma_start(c_out[s, 0:128, :], ct0[:])
        ct1 = drain.tile([128, D], F32, tag="ct")
        nc.scalar.activation(out=ct1[:, 0:512], in_=acc_a1[:], func=AF.Copy)
        nc.scalar.activation(out=ct1[:, 512:768], in_=acc_b1[:], func=AF.Copy)
        nc.sync.dma_start(c_out[s, 128:256, :], ct1[:])
        # c rows 256:384 and bounds rows 256:384 stay at the pre-zeroed output.


def build_program():
    nc = bacc.Bacc("TRN2", target_bir_lowering=False, debug=False,
                   enable_asserts=False, num_devices=NCORES)
    h_in = nc.dram_tensor("h_in", [SPC, T, D], F32, kind="ExternalInput").ap()
    a_in = nc.dram_tensor("alpha_in", [SPC, T], F32, kind="ExternalInput").ap()
    tl_in = nc.dram_tensor("tl_in", [SPC], I32, kind="ExternalInput").ap()
    c_out = nc.dram_tensor("c_out", [SPC, NMAX, D], F32, kind="ExternalOutput").ap()
    tl_out = nc.dram_tensor("tl_out", [SPC], I32, kind="ExternalOutput").ap()
    bd_out = nc.dram_tensor("bd_out", [SPC, NMAX, 2], I32,
                            kind="ExternalOutput").ap()
    with tile.TileContext(nc) as tc:
        with ExitStack() as ctx:
            _body(ctx, tc, (c_out, tl_out, bd_out), (h_in, a_in, tl_in))
    nc.compile()
    return nc


_nc_cache = None


def kernel(h, alpha, target_len, **_unused):
    global _nc_cache
    if _nc_cache is None:
        _nc_cache = build_program()
    nc = _nc_cache
    h = np.ascontiguousarray(np.asarray(h, dtype=np.float32))
    alpha = np.ascontiguousarray(np.asarray(alpha, dtype=np.float32))
    target_len = np.ascontiguousarray(np.asarray(target_len, dtype=np.int32))
    in_maps = []
    for i in range(NCORES):
        s0 = i * SPC
        in_maps.append({
            "h_in": np.ascontiguousarray(h[s0:s0 + SPC]),
            "alpha_in": np.ascontiguousarray(alpha[s0:s0 + SPC]),
            "tl_in": np.ascontiguousarray(target_len[s0:s0 + SPC]),
        })
    res = run_bass_kernel_spmd(nc, in_maps, core_ids=list(range(NCORES)))
    c = np.concatenate([r["c_out"] for r in res.results], axis=0)
    tl = np.concatenate([r["tl_out"] for r in res.results], axis=0)
    bd = np.concatenate([r["bd_out"] for r in res.results], axis=0)
    return c.astype(np.float32), tl.astype(np.int32), bd.astype(np.int32)


# revision 8
# speedup vs baseline: 1.0176x; 1.0176x over previous
"""CIF (continuous integrate-and-fire) Trainium2 Bass kernel.

Math: after scaling alpha so it sums to target_len, the CIF scan is a
segment-reduce: token k = sum_t W[k,t] * h[t] with
W[k,t] = |[k,k+1) ∩ [C_{t-1}, C_t)|  (C = cumsum of scaled alpha), i.e.
W[t->row, k->col] = clip01(C_t - k) - clip01(C_{t-1} - k).
Fire frame of token k: t_k = #{t : C_t < k+1}; boundaries[k] = [t_{k-1}, t_k+1],
tail token gets [t_{F-1}, T].  All computed on-device; batch is sharded
2 sequences per core across 8 cores.

Precision: C is computed two-level (per-125-frame scan + chunk-offset prefix),
with chunk totals split into an exact 2^-10-grid part and a tiny residual so
the offset prefix sums are exact; thresholds are evaluated as
(offset_hi - k) + offset_lo + C_local, keeping comparisons at ~1e-5 accuracy.
"""

import numpy as np
from contextlib import ExitStack

import concourse.bacc as bacc
import concourse.bass as bass
import concourse.tile as tile
from concourse import mybir
from concourse.bass_utils import run_bass_kernel_spmd

B, T, D = 16, 2000, 768
NCORES = 8
SPC = 2              # sequences per core
CHUNK = 125
NCHUNK = 16          # 2000 = 16 * 125
NTOK = 256           # max token index is 255 (target_len < 256)
NMAX = 384
MAGIC = 12582912.0   # 1.5 * 2^23: fp32 round-to-int via add/sub

F32 = mybir.dt.float32
F32R = mybir.dt.float32r
F16 = mybir.dt.float16
I32 = mybir.dt.int32
BF16 = mybir.dt.bfloat16
OP = mybir.AluOpType
AF = mybir.ActivationFunctionType


def _consts():
    lt32 = np.zeros((32, 32), np.float32)   # lt32[q,p]=1 iff same 16-block, q%16<p%16
    for q in range(32):
        for p in range(32):
            if q // 16 == p // 16 and q % 16 < p % 16:
                lt32[q, p] = 1.0
    sum32 = np.zeros((32, 2), np.float32)   # sum32[q,s]=1 iff q//16==s
    for q in range(32):
        sum32[q, q // 16] = 1.0
    id32 = np.eye(32, dtype=np.float32)
    return lt32, sum32, id32


def _bcast_ap(ap, n):
    """Partition-broadcast source AP for DMA: read the same row n times.

    `ap` must be a [1, m] (or [1]-leading) access pattern; the leading
    singleton is replaced by a zero-step dim of count n."""
    return bass.AP(tensor=ap.tensor, offset=ap.offset,
                   ap=[[0, n]] + [list(d) for d in ap.ap[1:]])


def _body(ctx, tc, outs, ins):
    nc = tc.nc
    c_out, tl_out, bd_out = outs
    h_in, a_in, tl_in = ins

    const = ctx.enter_context(tc.tile_pool(name="const", bufs=1))
    setup = ctx.enter_context(tc.tile_pool(name="setup", bufs=1))
    hpool = ctx.enter_context(tc.tile_pool(name="hp", bufs=6))
    work = ctx.enter_context(tc.tile_pool(name="work", bufs=5))
    drain = ctx.enter_context(tc.tile_pool(name="drain", bufs=2))
    fin = ctx.enter_context(tc.tile_pool(name="fin", bufs=2))
    ps_acc = ctx.enter_context(tc.tile_pool(name="psacc", bufs=1, space="PSUM"))
    ps_sm = ctx.enter_context(tc.tile_pool(name="pssm", bufs=2, space="PSUM"))

    # ---- static tiles (constants embedded in the NEFF) ----
    lt32_np, sum32_np, id32_np = _consts()
    lt32 = const.tile([32, 32], F32)
    nc.sync.dma_start(lt32[:], nc.inline_tensor(lt32_np, "lt32_c").ap())
    sum32 = const.tile([32, 2], F32)
    nc.sync.dma_start(sum32[:], nc.inline_tensor(sum32_np, "sum32_c").ap())
    id32 = const.tile([32, 32], F32)
    nc.sync.dma_start(id32[:], nc.inline_tensor(id32_np, "id32_c").ap())

    negio_i = const.tile([CHUNK, NTOK], I32)
    nc.gpsimd.iota(negio_i[:], pattern=[[1, NTOK]], base=0, channel_multiplier=0)
    negio = const.tile([CHUNK, NTOK], F32)
    nc.vector.tensor_scalar(out=negio[:], in0=negio_i[:], scalar1=-1.0,
                            scalar2=None, op0=OP.mult)
    kidx_i = const.tile([1, NTOK], I32)
    nc.gpsimd.iota(kidx_i[:], pattern=[[1, NTOK]], base=0, channel_multiplier=0)
    kidx = const.tile([1, NTOK], F32)
    nc.vector.tensor_copy(out=kidx[:], in_=kidx_i[:])
    ones_b = const.tile([CHUNK, 1], BF16)
    nc.vector.memset(ones_b[:], 1.0)

    # ---- per-core setup: scale, scan, offsets ----
    al = setup.tile([32, CHUNK], F32)
    nc.sync.dma_start(al[:], a_in.rearrange("s (c f) -> (s c) f", f=CHUNK))
    tl_i = setup.tile([SPC, 1], I32)
    nc.sync.dma_start(tl_i[:], tl_in.rearrange("(s one) -> s one", one=1))
    tlf = setup.tile([SPC, 1], F32)
    nc.vector.tensor_copy(out=tlf[:], in_=tl_i[:])

    tot_raw = setup.tile([32, 1], F32)
    nc.vector.reduce_sum(out=tot_raw[:], in_=al[:], axis=mybir.AxisListType.X)
    ps_fsr = ps_sm.tile([SPC, 1], F32, tag="sm")
    nc.tensor.matmul(ps_fsr[:], sum32[:], tot_raw[:], start=True, stop=True,
                     skip_group_check=True)
    # scale = tl / (sum + 1e-6), reciprocal + 1 Newton step
    dn = setup.tile([SPC, 1], F32)
    nc.vector.tensor_scalar(out=dn[:], in0=ps_fsr[:], scalar1=1e-6, scalar2=None,
                            op0=OP.add)
    r0 = setup.tile([SPC, 1], F32)
    nc.vector.reciprocal(out=r0[:], in_=dn[:])
    t0 = setup.tile([SPC, 1], F32)
    nc.vector.tensor_tensor(out=t0[:], in0=dn[:], in1=r0[:], op=OP.mult)
    u0 = setup.tile([SPC, 1], F32)
    nc.vector.tensor_scalar(out=u0[:], in0=t0[:], scalar1=-1.0, scalar2=2.0,
                            op0=OP.mult, op1=OP.add)
    r1n = setup.tile([SPC, 1], F32)
    nc.vector.tensor_tensor(out=r1n[:], in0=r0[:], in1=u0[:], op=OP.mult)
    scl = setup.tile([SPC, 1], F32)
    nc.vector.tensor_tensor(out=scl[:], in0=tlf[:], in1=r1n[:], op=OP.mult)
    sbc = setup.tile([32, 1], F32)
    scr_scl = nc.dram_tensor("scr_scl", [SPC, 1], F32, kind="Internal").ap()
    nc.sync.dma_start(scr_scl[:], scl[:])
    for s in range(SPC):
        row = scr_scl[s:s + 1, :]
        nc.gpsimd.dma_start(sbc[16 * s:16 * (s + 1), :], _bcast_ap(row, 16))

    als = setup.tile([32, CHUNK], F32)
    nc.vector.tensor_scalar(out=als[:], in0=al[:], scalar1=sbc[:], scalar2=None,
                            op0=OP.mult)
    cl = setup.tile([32, CHUNK], F32)
    nc.vector.tensor_tensor_scan(out=cl[:], data0=als[:], data1=als[:],
                                 initial=0.0, op0=OP.add, op1=OP.bypass)
    # exclusive chunk-offset prefix + per-seq final C
    tot = cl[:, CHUNK - 1:CHUNK]
    ps_off = ps_sm.tile([32, 1], F32, tag="sm")
    nc.tensor.matmul(ps_off[:], lt32[:], tot, start=True, stop=True,
                     skip_group_check=True)
    ps_fs = ps_sm.tile([SPC, 1], F32, tag="sm")
    nc.tensor.matmul(ps_fs[:], sum32[:], tot, start=True, stop=True,
                     skip_group_check=True)
    cfin2 = setup.tile([SPC, 1], F32)
    nc.vector.tensor_copy(out=cfin2[:], in_=ps_fs[:])
    ps_cfT = ps_sm.tile([1, SPC], F32, tag="sm")
    nc.tensor.transpose(ps_cfT[:], cfin2[:], id32[0:SPC, 0:SPC])
    cfinT = setup.tile([1, SPC], F32)
    nc.vector.tensor_copy(out=cfinT[:], in_=ps_cfT[:])

    off_sb = setup.tile([32, 1], F32)
    nc.vector.tensor_copy(out=off_sb[:], in_=ps_off[:])
    ps_o = ps_sm.tile([1, 32], F32, tag="sm")
    nc.tensor.transpose(ps_o[:], off_sb[:], id32[:])
    offT0 = setup.tile([1, 32], F32)
    nc.vector.tensor_copy(out=offT0[:], in_=ps_o[:])
    scr_off = nc.dram_tensor("scr_off", [1, 32], F32, kind="Internal").ap()
    nc.sync.dma_start(scr_off[:], offT0[:])
    offh = setup.tile([CHUNK, 32], F32)
    nc.gpsimd.dma_start(offh[:], _bcast_ap(scr_off[0:1, :], CHUNK))

    # transposed local-cumsum columns: CEcur[f,cidx]=Cl[cidx,f], CEprev shifted
    clp = setup.tile([32, CHUNK], F32)
    nc.vector.memset(clp[:, 0:1], 0.0)
    nc.vector.tensor_copy(out=clp[:, 1:CHUNK], in_=cl[:, 0:CHUNK - 1])
    ps_t1 = ps_sm.tile([CHUNK, 32], F32, tag="aux")
    nc.tensor.transpose(ps_t1[:], cl[:], id32[:])
    cecur = setup.tile([CHUNK, 32], F32)
    nc.vector.tensor_copy(out=cecur[:], in_=ps_t1[:])
    ps_t2 = ps_sm.tile([CHUNK, 32], F32, tag="aux")
    nc.tensor.transpose(ps_t2[:], clp[:], id32[:])
    ceprev = setup.tile([CHUNK, 32], F32)
    nc.vector.tensor_copy(out=ceprev[:], in_=ps_t2[:])
    # fold chunk offset into the ACT bias columns
    bias_a = setup.tile([CHUNK, 32], F32)
    nc.vector.tensor_tensor(out=bias_a[:], in0=cecur[:], in1=offh[:], op=OP.add)
    bias_b = setup.tile([CHUNK, 32], F32)
    nc.vector.tensor_tensor(out=bias_b[:], in0=ceprev[:], in1=offh[:], op=OP.add)

    # ---- main loop ----
    for s in range(SPC):
        counts_ps = ps_sm.tile([1, NTOK], F32, tag="aux")
        acc_a0 = ps_acc.tile([128, 512], F32, tag="a0")
        acc_b0 = ps_acc.tile([128, 256], F32, tag="b0")
        acc_a1 = ps_acc.tile([128, 512], F32, tag="a1")
        acc_b1 = ps_acc.tile([128, 256], F32, tag="b1")
        for c in range(NCHUNK):
            cidx = 16 * s + c
            ht = hpool.tile([CHUNK, D], F32, tag="h")
            nc.sync.dma_start(ht[:], h_in[s, c * CHUNK:(c + 1) * CHUNK, :])
            r1a = work.tile([CHUNK, NTOK], F32, tag="r1a")
            nc.scalar.activation(out=r1a[:], in_=negio[:], func=AF.Relu,
                                 bias=bias_a[:, cidx:cidx + 1], scale=1.0)
            r1b = work.tile([CHUNK, NTOK], F32, tag="r1b")
            nc.scalar.activation(out=r1b[:], in_=negio[:], func=AF.Relu,
                                 bias=bias_b[:, cidx:cidx + 1], scale=1.0)
            pb = work.tile([CHUNK, NTOK], F32, tag="pb")
            nc.vector.tensor_scalar(out=pb[:], in0=r1b[:], scalar1=1.0,
                                    scalar2=None, op0=OP.min)
            wt = work.tile([CHUNK, NTOK], F16, tag="w")
            nc.vector.scalar_tensor_tensor(out=wt[:], in0=r1a[:], scalar=1.0,
                                           in1=pb[:], op0=OP.min,
                                           op1=OP.subtract)
            ind = work.tile([CHUNK, NTOK], BF16, tag="ind")
            nc.vector.tensor_scalar(out=ind[:], in0=r1a[:], scalar1=1.0,
                                    scalar2=None, op0=OP.is_lt)
            hrr = work.tile([CHUNK, D], F16, tag="hr")
            nc.vector.tensor_copy(out=hrr[:, 0:512], in_=ht[:, 0:512])
            nc.scalar.copy(out=hrr[:, 512:768], in_=ht[:, 512:768])
            st, sp = (c == 0), (c == NCHUNK - 1)
            nc.tensor.matmul(counts_ps[:], ones_b[:], ind[:], start=st, stop=sp,
                             skip_group_check=True)
            nc.tensor.matmul(acc_a0[:], wt[:, 0:128], hrr[:, 0:512], start=st,
                             stop=sp, skip_group_check=True)
            nc.tensor.matmul(acc_b0[:], wt[:, 0:128], hrr[:, 512:768], start=st,
                             stop=sp, skip_group_check=True)
            nc.tensor.matmul(acc_a1[:], wt[:, 128:256], hrr[:, 0:512], start=st,
                             stop=sp, skip_group_check=True)
            nc.tensor.matmul(acc_b1[:], wt[:, 128:256], hrr[:, 512:768], start=st,
                             stop=sp, skip_group_check=True)

        # ---- finalize sequence s ----
        csb = fin.tile([1, NTOK], F32, tag="csb")
        nc.vector.tensor_copy(out=csb[:], in_=counts_ps[:])
        ltc = fin.tile([1, NTOK], F32, tag="ltc")
        nc.vector.tensor_scalar(out=ltc[:], in0=csb[:], scalar1=float(T),
                                scalar2=None, op0=OP.is_lt)
        ft = fin.tile([1, 1], F32, tag="ft")
        nc.vector.reduce_sum(out=ft[:], in_=ltc[:], axis=mybir.AxisListType.X)
        frac = fin.tile([1, 1], F32, tag="frac")
        nc.vector.tensor_tensor(out=frac[:], in0=cfinT[:, s:s + 1], in1=ft[:],
                                op=OP.subtract)
        tailf = fin.tile([1, 1], F32, tag="tailf")
        nc.vector.tensor_scalar(out=tailf[:], in0=frac[:], scalar1=1e-4,
                                scalar2=None, op0=OP.is_gt)
        ntokf = fin.tile([1, 1], F32, tag="ntokf")
        nc.vector.tensor_tensor(out=ntokf[:], in0=ft[:], in1=tailf[:], op=OP.add)
        tli = fin.tile([1, 1], I32, tag="tli")
        nc.vector.tensor_copy(out=tli[:], in_=ntokf[:])
        nc.sync.dma_start(tl_out[s:s + 1].rearrange("(a b) -> a b", b=1), tli[:])

        valid = fin.tile([1, NTOK], F32, tag="valid")
        nc.vector.tensor_scalar(out=valid[:], in0=kidx[:], scalar1=ntokf[:],
                                scalar2=None, op0=OP.is_lt)
        isfire = fin.tile([1, NTOK], F32, tag="isfire")
        nc.vector.tensor_scalar(out=isfire[:], in0=kidx[:], scalar1=ft[:],
                                scalar2=None, op0=OP.is_lt)
        c1 = fin.tile([1, NTOK], F32, tag="c1")
        nc.vector.tensor_scalar(out=c1[:], in0=csb[:], scalar1=1.0, scalar2=None,
                                op0=OP.add)
        endsA = fin.tile([1, NTOK], F32, tag="endsA")
        nc.vector.tensor_tensor(out=endsA[:], in0=c1[:], in1=isfire[:], op=OP.mult)
        tailm = fin.tile([1, NTOK], F32, tag="tailm")
        nc.vector.tensor_tensor(out=tailm[:], in0=valid[:], in1=isfire[:],
                                op=OP.subtract)
        ends = fin.tile([1, NTOK], F32, tag="ends")
        nc.vector.scalar_tensor_tensor(out=ends[:], in0=tailm[:],
                                       scalar=float(T), in1=endsA[:],
                                       op0=OP.mult, op1=OP.add)
        ssh = fin.tile([1, NTOK], F32, tag="ssh")
        nc.vector.memset(ssh[:, 0:1], 0.0)
        nc.vector.tensor_copy(out=ssh[:, 1:NTOK], in_=csb[:, 0:NTOK - 1])
        starts = fin.tile([1, NTOK], F32, tag="starts")
        nc.vector.tensor_tensor(out=starts[:], in0=ssh[:], in1=valid[:],
                                op=OP.mult)
        ends_i = fin.tile([1, NTOK], I32, tag="ends_i")
        nc.vector.tensor_copy(out=ends_i[:], in_=ends[:])
        starts_i = fin.tile([1, NTOK], I32, tag="starts_i")
        nc.vector.tensor_copy(out=starts_i[:], in_=starts[:])
        nc.sync.dma_start(
            bd_out[s, 0:NTOK, 0].rearrange("(one n) -> one n", one=1), starts_i[:])
        nc.sync.dma_start(
            bd_out[s, 0:NTOK, 1].rearrange("(one n) -> one n", one=1), ends_i[:])

        ct0 = drain.tile([128, D], F32, tag="ct")
        nc.scalar.activation(out=ct0[:, 0:512], in_=acc_a0[:], func=AF.Copy)
        nc.scalar.activation(out=ct0[:, 512:768], in_=acc_b0[:], func=AF.Copy)
        nc.sync.dma_start(c_out[s, 0:128, :], ct0[:])
        ct1 = drain.tile([128, D], F32, tag="ct")
        nc.scalar.activation(out=ct1[:, 0:512], in_=acc_a1[:], func=AF.Copy)
        nc.scalar.activation(out=ct1[:, 512:768], in_=acc_b1[:], func=AF.Copy)
        nc.sync.dma_start(c_out[s, 128:256, :], ct1[:])
        # c rows 256:384 and bounds rows 256:384 stay at the pre-zeroed output.


def build_program():
    nc = bacc.Bacc("TRN2", target_bir_lowering=False, debug=False,
                   enable_asserts=False, num_devices=NCORES)
    h_in = nc.dram_tensor("h_in", [SPC, T, D], F32, kind="ExternalInput").ap()
    a_in = nc.dram_tensor("alpha_in", [SPC, T], F32, kind="ExternalInput").ap()
    tl_in = nc.dram_tensor("tl_in", [SPC], I32, kind="ExternalInput").ap()
    c_out = nc.dram_tensor("c_out", [SPC, NMAX, D], F32, kind="ExternalOutput").ap()
    tl_out = nc.dram_tensor("tl_out", [SPC], I32, kind="ExternalOutput").ap()
    bd_out = nc.dram_tensor("bd_out", [SPC, NMAX, 2], I32,
                            kind="ExternalOutput").ap()
    with tile.TileContext(nc) as tc:
        with ExitStack() as ctx:
            _body(ctx, tc, (c_out, tl_out, bd_out), (h_in, a_in, tl_in))
    nc.compile()
    return nc


_nc_cache = None


def kernel(h, alpha, target_len, **_unused):
    global _nc_cache
    if _nc_cache is None:
        _nc_cache = build_program()
    nc = _nc_cache
    h = np.ascontiguousarray(np.asarray(h, dtype=np.float32))
    alpha = np.ascontiguousarray(np.asarray(alpha, dtype=np.float32))
    target_len = np.ascontiguousarray(np.asarray(target_len, dtype=np.int32))
    in_maps = []
    for i in range(NCORES):
        s0 = i * SPC
        in_maps.append({
            "h_in": np.ascontiguousarray(h[s0:s0 + SPC]),
            "alpha_in": np.ascontiguousarray(alpha[s0:s0 + SPC]),
            "tl_in": np.ascontiguousarray(target_len[s0:s0 + SPC]),
        })
    res = run_bass_kernel_spmd(nc, in_maps, core_ids=list(range(NCORES)))
    c = np.concatenate([r["c_out"] for r in res.results], axis=0)
    tl = np.concatenate([r["tl_out"] for r in res.results], axis=0)
    bd = np.concatenate([r["bd_out"] for r in res.results], axis=0)
    return c.astype(np.float32), tl.astype(np.int32), bd.astype(np.int32)


# revision 9
# speedup vs baseline: 1.3206x; 1.2977x over previous
"""CIF (continuous integrate-and-fire) Trainium2 Bass kernel.

Math: after scaling alpha so it sums to target_len, the CIF scan is a
segment-reduce: token k = sum_t W[k,t] * h[t] with
W[k,t] = |[k,k+1) ∩ [C_{t-1}, C_t)|  (C = cumsum of scaled alpha), i.e.
W[t->row, k->col] = clip01(C_t - k) - clip01(C_{t-1} - k).
Fire frame of token k: t_k = #{t : C_t < k+1}; boundaries[k] = [t_{k-1}, t_k+1],
tail token gets [t_{F-1}, T].  All computed on-device; batch is sharded
2 sequences per core across 8 cores.

Precision: C is computed two-level (per-125-frame scan + chunk-offset prefix),
with chunk totals split into an exact 2^-10-grid part and a tiny residual so
the offset prefix sums are exact; thresholds are evaluated as
(offset_hi - k) + offset_lo + C_local, keeping comparisons at ~1e-5 accuracy.
"""

import numpy as np
from contextlib import ExitStack

import concourse.bacc as bacc
import concourse.bass as bass
import concourse.tile as tile
from concourse import mybir
from concourse.bass_utils import run_bass_kernel_spmd

B, T, D = 16, 2000, 768
NCORES = 8
SPC = 2              # sequences per core
CHUNK = 125
NCHUNK = 16          # 2000 = 16 * 125
NTOK = 256           # max token index is 255 (target_len < 256)
NMAX = 384
MAGIC = 12582912.0   # 1.5 * 2^23: fp32 round-to-int via add/sub

F32 = mybir.dt.float32
F32R = mybir.dt.float32r
F16 = mybir.dt.float16
I32 = mybir.dt.int32
BF16 = mybir.dt.bfloat16
OP = mybir.AluOpType
AF = mybir.ActivationFunctionType


def _consts():
    lt32 = np.zeros((32, 32), np.float32)   # lt32[q,p]=1 iff same 16-block, q%16<p%16
    for q in range(32):
        for p in range(32):
            if q // 16 == p // 16 and q % 16 < p % 16:
                lt32[q, p] = 1.0
    sum32 = np.zeros((32, 2), np.float32)   # sum32[q,s]=1 iff q//16==s
    for q in range(32):
        sum32[q, q // 16] = 1.0
    id32 = np.eye(32, dtype=np.float32)
    return lt32, sum32, id32


def _bcast_ap(ap, n):
    """Partition-broadcast source AP for DMA: read the same row n times.

    `ap` must be a [1, m] (or [1]-leading) access pattern; the leading
    singleton is replaced by a zero-step dim of count n."""
    return bass.AP(tensor=ap.tensor, offset=ap.offset,
                   ap=[[0, n]] + [list(d) for d in ap.ap[1:]])


def _body(ctx, tc, outs, ins):
    nc = tc.nc
    c_out, tl_out, bd_out = outs
    h_in, a_in, tl_in = ins

    const = ctx.enter_context(tc.tile_pool(name="const", bufs=1))
    setup = ctx.enter_context(tc.tile_pool(name="setup", bufs=1))
    hpool = ctx.enter_context(tc.tile_pool(name="hp", bufs=6))
    work = ctx.enter_context(tc.tile_pool(name="work", bufs=5))
    drain = ctx.enter_context(tc.tile_pool(name="drain", bufs=2))
    fin = ctx.enter_context(tc.tile_pool(name="fin", bufs=2))
    ps_acc = ctx.enter_context(tc.tile_pool(name="psacc", bufs=1, space="PSUM"))
    ps_sm = ctx.enter_context(tc.tile_pool(name="pssm", bufs=2, space="PSUM"))

    # ---- static tiles (constants embedded in the NEFF) ----
    lt32_np, sum32_np, id32_np = _consts()
    lt32 = const.tile([32, 32], F32)
    nc.sync.dma_start(lt32[:], nc.inline_tensor(lt32_np, "lt32_c").ap())
    sum32 = const.tile([32, 2], F32)
    nc.sync.dma_start(sum32[:], nc.inline_tensor(sum32_np, "sum32_c").ap())
    id32 = const.tile([32, 32], F32)
    nc.sync.dma_start(id32[:], nc.inline_tensor(id32_np, "id32_c").ap())

    negio_i = const.tile([CHUNK, NTOK], I32)
    nc.gpsimd.iota(negio_i[:], pattern=[[1, NTOK]], base=0, channel_multiplier=0)
    negio = const.tile([CHUNK, NTOK], F32)
    nc.vector.tensor_scalar(out=negio[:], in0=negio_i[:], scalar1=-1.0,
                            scalar2=None, op0=OP.mult)
    kidx_i = const.tile([1, NTOK], I32)
    nc.gpsimd.iota(kidx_i[:], pattern=[[1, NTOK]], base=0, channel_multiplier=0)
    kidx = const.tile([1, NTOK], F32)
    nc.vector.tensor_copy(out=kidx[:], in_=kidx_i[:])
    ones_b = const.tile([CHUNK, 1], BF16)
    nc.vector.memset(ones_b[:], 1.0)

    # ---- per-core setup: scale, scan, offsets ----
    al = setup.tile([32, CHUNK], F32)
    nc.sync.dma_start(al[:], a_in.rearrange("s (c f) -> (s c) f", f=CHUNK))
    tl_i = setup.tile([SPC, 1], I32)
    nc.sync.dma_start(tl_i[:], tl_in.rearrange("(s one) -> s one", one=1))
    tlf = setup.tile([SPC, 1], F32)
    nc.vector.tensor_copy(out=tlf[:], in_=tl_i[:])

    tot_raw = setup.tile([32, 1], F32)
    nc.vector.reduce_sum(out=tot_raw[:], in_=al[:], axis=mybir.AxisListType.X)
    ps_fsr = ps_sm.tile([SPC, 1], F32, tag="sm")
    nc.tensor.matmul(ps_fsr[:], sum32[:], tot_raw[:], start=True, stop=True,
                     skip_group_check=True)
    # scale = tl / (sum + 1e-6), reciprocal + 1 Newton step
    dn = setup.tile([SPC, 1], F32)
    nc.vector.tensor_scalar(out=dn[:], in0=ps_fsr[:], scalar1=1e-6, scalar2=None,
                            op0=OP.add)
    r0 = setup.tile([SPC, 1], F32)
    nc.vector.reciprocal(out=r0[:], in_=dn[:])
    t0 = setup.tile([SPC, 1], F32)
    nc.vector.tensor_tensor(out=t0[:], in0=dn[:], in1=r0[:], op=OP.mult)
    u0 = setup.tile([SPC, 1], F32)
    nc.vector.tensor_scalar(out=u0[:], in0=t0[:], scalar1=-1.0, scalar2=2.0,
                            op0=OP.mult, op1=OP.add)
    r1n = setup.tile([SPC, 1], F32)
    nc.vector.tensor_tensor(out=r1n[:], in0=r0[:], in1=u0[:], op=OP.mult)
    scl = setup.tile([SPC, 1], F32)
    nc.vector.tensor_tensor(out=scl[:], in0=tlf[:], in1=r1n[:], op=OP.mult)
    sbc = setup.tile([32, 1], F32)
    scr_scl = nc.dram_tensor("scr_scl", [SPC, 1], F32, kind="Internal").ap()
    nc.sync.dma_start(scr_scl[:], scl[:])
    for s in range(SPC):
        row = scr_scl[s:s + 1, :]
        nc.gpsimd.dma_start(sbc[16 * s:16 * (s + 1), :], _bcast_ap(row, 16))

    als = setup.tile([32, CHUNK], F32)
    nc.vector.tensor_scalar(out=als[:], in0=al[:], scalar1=sbc[:], scalar2=None,
                            op0=OP.mult)
    cl = setup.tile([32, CHUNK], F32)
    nc.vector.tensor_tensor_scan(out=cl[:], data0=als[:], data1=als[:],
                                 initial=0.0, op0=OP.add, op1=OP.bypass)
    # exclusive chunk-offset prefix + per-seq final C
    tot = cl[:, CHUNK - 1:CHUNK]
    ps_off = ps_sm.tile([32, 1], F32, tag="sm")
    nc.tensor.matmul(ps_off[:], lt32[:], tot, start=True, stop=True,
                     skip_group_check=True)
    ps_fs = ps_sm.tile([SPC, 1], F32, tag="sm")
    nc.tensor.matmul(ps_fs[:], sum32[:], tot, start=True, stop=True,
                     skip_group_check=True)
    cfin2 = setup.tile([SPC, 1], F32)
    nc.vector.tensor_copy(out=cfin2[:], in_=ps_fs[:])
    ps_cfT = ps_sm.tile([1, SPC], F32, tag="sm")
    nc.tensor.transpose(ps_cfT[:], cfin2[:], id32[0:SPC, 0:SPC])
    cfinT = setup.tile([1, SPC], F32)
    nc.vector.tensor_copy(out=cfinT[:], in_=ps_cfT[:])

    off_sb = setup.tile([32, 1], F32)
    nc.vector.tensor_copy(out=off_sb[:], in_=ps_off[:])
    ps_o = ps_sm.tile([1, 32], F32, tag="sm")
    nc.tensor.transpose(ps_o[:], off_sb[:], id32[:])
    offT0 = setup.tile([1, 32], F32)
    nc.vector.tensor_copy(out=offT0[:], in_=ps_o[:])
    scr_off = nc.dram_tensor("scr_off", [1, 32], F32, kind="Internal").ap()
    nc.sync.dma_start(scr_off[:], offT0[:])
    offh = setup.tile([CHUNK, 32], F32)
    nc.gpsimd.dma_start(offh[:], _bcast_ap(scr_off[0:1, :], CHUNK))

    # transposed local-cumsum columns: CEcur[f,cidx]=Cl[cidx,f], CEprev shifted
    clp = setup.tile([32, CHUNK], F32)
    nc.vector.memset(clp[:, 0:1], 0.0)
    nc.vector.tensor_copy(out=clp[:, 1:CHUNK], in_=cl[:, 0:CHUNK - 1])
    ps_t1 = ps_sm.tile([CHUNK, 32], F32, tag="aux")
    nc.tensor.transpose(ps_t1[:], cl[:], id32[:])
    cecur = setup.tile([CHUNK, 32], F32)
    nc.vector.tensor_copy(out=cecur[:], in_=ps_t1[:])
    ps_t2 = ps_sm.tile([CHUNK, 32], F32, tag="aux")
    nc.tensor.transpose(ps_t2[:], clp[:], id32[:])
    ceprev = setup.tile([CHUNK, 32], F32)
    nc.vector.tensor_copy(out=ceprev[:], in_=ps_t2[:])
    # fold chunk offset into the ACT bias columns
    bias_a = setup.tile([CHUNK, 32], F32)
    nc.vector.tensor_tensor(out=bias_a[:], in0=cecur[:], in1=offh[:], op=OP.add)
    bias_b = setup.tile([CHUNK, 32], F32)
    nc.vector.tensor_tensor(out=bias_b[:], in0=ceprev[:], in1=offh[:], op=OP.add)

    # ---- main loop ----
    for s in range(SPC):
        counts_ps = ps_sm.tile([1, NTOK], F32, tag="aux")
        acc_a0 = ps_acc.tile([128, 512], F32, tag="a0")
        acc_b0 = ps_acc.tile([128, 256], F32, tag="b0")
        acc_a1 = ps_acc.tile([128, 512], F32, tag="a1")
        acc_b1 = ps_acc.tile([128, 256], F32, tag="b1")
        for c in range(NCHUNK):
            cidx = 16 * s + c
            ht = hpool.tile([CHUNK, D], F32, tag="h")
            nc.gpsimd.dma_start(ht[:], h_in[s, c * CHUNK:(c + 1) * CHUNK, :])
            r1a = work.tile([CHUNK, NTOK], F32, tag="r1a")
            nc.scalar.activation(out=r1a[:], in_=negio[:], func=AF.Relu,
                                 bias=bias_a[:, cidx:cidx + 1], scale=1.0)
            r1b = work.tile([CHUNK, NTOK], F32, tag="r1b")
            nc.scalar.activation(out=r1b[:], in_=negio[:], func=AF.Relu,
                                 bias=bias_b[:, cidx:cidx + 1], scale=1.0)
            pb = work.tile([CHUNK, NTOK], F32, tag="pb")
            nc.vector.tensor_scalar(out=pb[:], in0=r1b[:], scalar1=1.0,
                                    scalar2=None, op0=OP.min)
            wt = work.tile([CHUNK, NTOK], F16, tag="w")
            nc.vector.scalar_tensor_tensor(out=wt[:], in0=r1a[:], scalar=1.0,
                                           in1=pb[:], op0=OP.min,
                                           op1=OP.subtract)
            ind = work.tile([CHUNK, NTOK], BF16, tag="ind")
            nc.vector.tensor_scalar(out=ind[:], in0=r1a[:], scalar1=1.0,
                                    scalar2=None, op0=OP.is_lt)
            hrr = work.tile([CHUNK, D], F16, tag="hr")
            nc.vector.tensor_copy(out=hrr[:, 0:512], in_=ht[:, 0:512])
            nc.scalar.copy(out=hrr[:, 512:768], in_=ht[:, 512:768])
            st, sp = (c == 0), (c == NCHUNK - 1)
            nc.tensor.matmul(counts_ps[:], ones_b[:], ind[:], start=st, stop=sp,
                             skip_group_check=True)
            nc.tensor.matmul(acc_a0[:], wt[:, 0:128], hrr[:, 0:512], start=st,
                             stop=sp, skip_group_check=True)
            nc.tensor.matmul(acc_b0[:], wt[:, 0:128], hrr[:, 512:768], start=st,
                             stop=sp, skip_group_check=True)
            nc.tensor.matmul(acc_a1[:], wt[:, 128:256], hrr[:, 0:512], start=st,
                             stop=sp, skip_group_check=True)
            nc.tensor.matmul(acc_b1[:], wt[:, 128:256], hrr[:, 512:768], start=st,
                             stop=sp, skip_group_check=True)

        # ---- finalize sequence s ----
        csb = fin.tile([1, NTOK], F32, tag="csb")
        nc.vector.tensor_copy(out=csb[:], in_=counts_ps[:])
        ltc = fin.tile([1, NTOK], F32, tag="ltc")
        nc.vector.tensor_scalar(out=ltc[:], in0=csb[:], scalar1=float(T),
                                scalar2=None, op0=OP.is_lt)
        ft = fin.tile([1, 1], F32, tag="ft")
        nc.vector.reduce_sum(out=ft[:], in_=ltc[:], axis=mybir.AxisListType.X)
        frac = fin.tile([1, 1], F32, tag="frac")
        nc.vector.tensor_tensor(out=frac[:], in0=cfinT[:, s:s + 1], in1=ft[:],
                                op=OP.subtract)
        tailf = fin.tile([1, 1], F32, tag="tailf")
        nc.vector.tensor_scalar(out=tailf[:], in0=frac[:], scalar1=1e-4,
                                scalar2=None, op0=OP.is_gt)
        ntokf = fin.tile([1, 1], F32, tag="ntokf")
        nc.vector.tensor_tensor(out=ntokf[:], in0=ft[:], in1=tailf[:], op=OP.add)
        tli = fin.tile([1, 1], I32, tag="tli")
        nc.vector.tensor_copy(out=tli[:], in_=ntokf[:])
        nc.sync.dma_start(tl_out[s:s + 1].rearrange("(a b) -> a b", b=1), tli[:])

        valid = fin.tile([1, NTOK], F32, tag="valid")
        nc.vector.tensor_scalar(out=valid[:], in0=kidx[:], scalar1=ntokf[:],
                                scalar2=None, op0=OP.is_lt)
        isfire = fin.tile([1, NTOK], F32, tag="isfire")
        nc.vector.tensor_scalar(out=isfire[:], in0=kidx[:], scalar1=ft[:],
                                scalar2=None, op0=OP.is_lt)
        c1 = fin.tile([1, NTOK], F32, tag="c1")
        nc.vector.tensor_scalar(out=c1[:], in0=csb[:], scalar1=1.0, scalar2=None,
                                op0=OP.add)
        endsA = fin.tile([1, NTOK], F32, tag="endsA")
        nc.vector.tensor_tensor(out=endsA[:], in0=c1[:], in1=isfire[:], op=OP.mult)
        tailm = fin.tile([1, NTOK], F32, tag="tailm")
        nc.vector.tensor_tensor(out=tailm[:], in0=valid[:], in1=isfire[:],
                                op=OP.subtract)
        ends = fin.tile([1, NTOK], F32, tag="ends")
        nc.vector.scalar_tensor_tensor(out=ends[:], in0=tailm[:],
                                       scalar=float(T), in1=endsA[:],
                                       op0=OP.mult, op1=OP.add)
        ssh = fin.tile([1, NTOK], F32, tag="ssh")
        nc.vector.memset(ssh[:, 0:1], 0.0)
        nc.vector.tensor_copy(out=ssh[:, 1:NTOK], in_=csb[:, 0:NTOK - 1])
        starts = fin.tile([1, NTOK], F32, tag="starts")
        nc.vector.tensor_tensor(out=starts[:], in0=ssh[:], in1=valid[:],
                                op=OP.mult)
        ends_i = fin.tile([1, NTOK], I32, tag="ends_i")
        nc.vector.tensor_copy(out=ends_i[:], in_=ends[:])
        starts_i = fin.tile([1, NTOK], I32, tag="starts_i")
        nc.vector.tensor_copy(out=starts_i[:], in_=starts[:])
        nc.sync.dma_start(
            bd_out[s, 0:NTOK, 0].rearrange("(one n) -> one n", one=1), starts_i[:])
        nc.sync.dma_start(
            bd_out[s, 0:NTOK, 1].rearrange("(one n) -> one n", one=1), ends_i[:])

        ct0 = drain.tile([128, D], F32, tag="ct")
        nc.scalar.activation(out=ct0[:, 0:512], in_=acc_a0[:], func=AF.Copy)
        nc.scalar.activation(out=ct0[:, 512:768], in_=acc_b0[:], func=AF.Copy)
        nc.sync.dma_start(c_out[s, 0:128, :], ct0[:])
        ct1 = drain.tile([128, D], F32, tag="ct")
        nc.scalar.activation(out=ct1[:, 0:512], in_=acc_a1[:], func=AF.Copy)
        nc.scalar.activation(out=ct1[:, 512:768], in_=acc_b1[:], func=AF.Copy)
        nc.sync.dma_start(c_out[s, 128:256, :], ct1[:])
        # c rows 256:384 and bounds rows 256:384 stay at the pre-zeroed output.


def build_program():
    nc = bacc.Bacc("TRN2", target_bir_lowering=False, debug=False,
                   enable_asserts=False, num_devices=NCORES)
    h_in = nc.dram_tensor("h_in", [SPC, T, D], F32, kind="ExternalInput").ap()
    a_in = nc.dram_tensor("alpha_in", [SPC, T], F32, kind="ExternalInput").ap()
    tl_in = nc.dram_tensor("tl_in", [SPC], I32, kind="ExternalInput").ap()
    c_out = nc.dram_tensor("c_out", [SPC, NMAX, D], F32, kind="ExternalOutput").ap()
    tl_out = nc.dram_tensor("tl_out", [SPC], I32, kind="ExternalOutput").ap()
    bd_out = nc.dram_tensor("bd_out", [SPC, NMAX, 2], I32,
                            kind="ExternalOutput").ap()
    with tile.TileContext(nc) as tc:
        with ExitStack() as ctx:
            _body(ctx, tc, (c_out, tl_out, bd_out), (h_in, a_in, tl_in))
    nc.compile()
    return nc


_nc_cache = None


def kernel(h, alpha, target_len, **_unused):
    global _nc_cache
    if _nc_cache is None:
        _nc_cache = build_program()
    nc = _nc_cache
    h = np.ascontiguousarray(np.asarray(h, dtype=np.float32))
    alpha = np.ascontiguousarray(np.asarray(alpha, dtype=np.float32))
    target_len = np.ascontiguousarray(np.asarray(target_len, dtype=np.int32))
    in_maps = []
    for i in range(NCORES):
        s0 = i * SPC
        in_maps.append({
            "h_in": np.ascontiguousarray(h[s0:s0 + SPC]),
            "alpha_in": np.ascontiguousarray(alpha[s0:s0 + SPC]),
            "tl_in": np.ascontiguousarray(target_len[s0:s0 + SPC]),
        })
    res = run_bass_kernel_spmd(nc, in_maps, core_ids=list(range(NCORES)))
    c = np.concatenate([r["c_out"] for r in res.results], axis=0)
    tl = np.concatenate([r["tl_out"] for r in res.results], axis=0)
    bd = np.concatenate([r["bd_out"] for r in res.results], axis=0)
    return c.astype(np.float32), tl.astype(np.int32), bd.astype(np.int32)


# revision 10
# speedup vs baseline: 1.3254x; 1.0036x over previous
"""CIF (continuous integrate-and-fire) Trainium2 Bass kernel.

Math: after scaling alpha so it sums to target_len, the CIF scan is a
segment-reduce: token k = sum_t W[k,t] * h[t] with
W[k,t] = |[k,k+1) ∩ [C_{t-1}, C_t)|  (C = cumsum of scaled alpha), i.e.
W[t->row, k->col] = clip01(C_t - k) - clip01(C_{t-1} - k).
Fire frame of token k: t_k = #{t : C_t < k+1}; boundaries[k] = [t_{k-1}, t_k+1],
tail token gets [t_{F-1}, T].  All computed on-device; batch is sharded
2 sequences per core across 8 cores.

Precision: C is computed two-level (per-125-frame scan + chunk-offset prefix),
with chunk totals split into an exact 2^-10-grid part and a tiny residual so
the offset prefix sums are exact; thresholds are evaluated as
(offset_hi - k) + offset_lo + C_local, keeping comparisons at ~1e-5 accuracy.
"""

import numpy as np
from contextlib import ExitStack

import concourse.bacc as bacc
import concourse.bass as bass
import concourse.tile as tile
from concourse import mybir
from concourse.bass_utils import run_bass_kernel_spmd

B, T, D = 16, 2000, 768
NCORES = 8
SPC = 2              # sequences per core
CHUNK = 125
NCHUNK = 16          # 2000 = 16 * 125
NTOK = 256           # max token index is 255 (target_len < 256)
NMAX = 384
MAGIC = 12582912.0   # 1.5 * 2^23: fp32 round-to-int via add/sub

F32 = mybir.dt.float32
F32R = mybir.dt.float32r
F16 = mybir.dt.float16
I32 = mybir.dt.int32
BF16 = mybir.dt.bfloat16
OP = mybir.AluOpType
AF = mybir.ActivationFunctionType


def _consts():
    lt32 = np.zeros((32, 32), np.float32)   # lt32[q,p]=1 iff same 16-block, q%16<p%16
    for q in range(32):
        for p in range(32):
            if q // 16 == p // 16 and q % 16 < p % 16:
                lt32[q, p] = 1.0
    sum32 = np.zeros((32, 2), np.float32)   # sum32[q,s]=1 iff q//16==s
    for q in range(32):
        sum32[q, q // 16] = 1.0
    id32 = np.eye(32, dtype=np.float32)
    return lt32, sum32, id32


def _bcast_ap(ap, n):
    """Partition-broadcast source AP for DMA: read the same row n times.

    `ap` must be a [1, m] (or [1]-leading) access pattern; the leading
    singleton is replaced by a zero-step dim of count n."""
    return bass.AP(tensor=ap.tensor, offset=ap.offset,
                   ap=[[0, n]] + [list(d) for d in ap.ap[1:]])


def _body(ctx, tc, outs, ins):
    nc = tc.nc
    c_out, tl_out, bd_out = outs
    h_in, a_in, tl_in = ins

    const = ctx.enter_context(tc.tile_pool(name="const", bufs=1))
    setup = ctx.enter_context(tc.tile_pool(name="setup", bufs=1))
    hpool = ctx.enter_context(tc.tile_pool(name="hp", bufs=6))
    work = ctx.enter_context(tc.tile_pool(name="work", bufs=6))
    drain = ctx.enter_context(tc.tile_pool(name="drain", bufs=2))
    fin = ctx.enter_context(tc.tile_pool(name="fin", bufs=2))
    ps_acc = ctx.enter_context(tc.tile_pool(name="psacc", bufs=1, space="PSUM"))
    ps_sm = ctx.enter_context(tc.tile_pool(name="pssm", bufs=2, space="PSUM"))

    # ---- static tiles (constants embedded in the NEFF) ----
    lt32_np, sum32_np, id32_np = _consts()
    lt32 = const.tile([32, 32], F32)
    nc.sync.dma_start(lt32[:], nc.inline_tensor(lt32_np, "lt32_c").ap())
    sum32 = const.tile([32, 2], F32)
    nc.sync.dma_start(sum32[:], nc.inline_tensor(sum32_np, "sum32_c").ap())
    id32 = const.tile([32, 32], F32)
    nc.sync.dma_start(id32[:], nc.inline_tensor(id32_np, "id32_c").ap())

    negio_i = const.tile([CHUNK, NTOK], I32)
    nc.gpsimd.iota(negio_i[:], pattern=[[1, NTOK]], base=0, channel_multiplier=0)
    negio = const.tile([CHUNK, NTOK], F32)
    nc.vector.tensor_scalar(out=negio[:], in0=negio_i[:], scalar1=-1.0,
                            scalar2=None, op0=OP.mult)
    kidx_i = const.tile([1, NTOK], I32)
    nc.gpsimd.iota(kidx_i[:], pattern=[[1, NTOK]], base=0, channel_multiplier=0)
    kidx = const.tile([1, NTOK], F32)
    nc.vector.tensor_copy(out=kidx[:], in_=kidx_i[:])
    ones_b = const.tile([CHUNK, 1], BF16)
    nc.vector.memset(ones_b[:], 1.0)

    # ---- per-core setup: scale, scan, offsets ----
    al = setup.tile([32, CHUNK], F32)
    nc.sync.dma_start(al[:], a_in.rearrange("s (c f) -> (s c) f", f=CHUNK))
    tl_i = setup.tile([SPC, 1], I32)
    nc.sync.dma_start(tl_i[:], tl_in.rearrange("(s one) -> s one", one=1))
    tlf = setup.tile([SPC, 1], F32)
    nc.vector.tensor_copy(out=tlf[:], in_=tl_i[:])

    tot_raw = setup.tile([32, 1], F32)
    nc.vector.reduce_sum(out=tot_raw[:], in_=al[:], axis=mybir.AxisListType.X)
    ps_fsr = ps_sm.tile([SPC, 1], F32, tag="sm")
    nc.tensor.matmul(ps_fsr[:], sum32[:], tot_raw[:], start=True, stop=True,
                     skip_group_check=True)
    # scale = tl / (sum + 1e-6), reciprocal + 1 Newton step
    dn = setup.tile([SPC, 1], F32)
    nc.vector.tensor_scalar(out=dn[:], in0=ps_fsr[:], scalar1=1e-6, scalar2=None,
                            op0=OP.add)
    r0 = setup.tile([SPC, 1], F32)
    nc.vector.reciprocal(out=r0[:], in_=dn[:])
    t0 = setup.tile([SPC, 1], F32)
    nc.vector.tensor_tensor(out=t0[:], in0=dn[:], in1=r0[:], op=OP.mult)
    u0 = setup.tile([SPC, 1], F32)
    nc.vector.tensor_scalar(out=u0[:], in0=t0[:], scalar1=-1.0, scalar2=2.0,
                            op0=OP.mult, op1=OP.add)
    r1n = setup.tile([SPC, 1], F32)
    nc.vector.tensor_tensor(out=r1n[:], in0=r0[:], in1=u0[:], op=OP.mult)
    scl = setup.tile([SPC, 1], F32)
    nc.vector.tensor_tensor(out=scl[:], in0=tlf[:], in1=r1n[:], op=OP.mult)
    sbc = setup.tile([32, 1], F32)
    scr_scl = nc.dram_tensor("scr_scl", [SPC, 1], F32, kind="Internal").ap()
    nc.sync.dma_start(scr_scl[:], scl[:])
    for s in range(SPC):
        row = scr_scl[s:s + 1, :]
        nc.gpsimd.dma_start(sbc[16 * s:16 * (s + 1), :], _bcast_ap(row, 16))

    als = setup.tile([32, CHUNK], F32)
    nc.vector.tensor_scalar(out=als[:], in0=al[:], scalar1=sbc[:], scalar2=None,
                            op0=OP.mult)
    cl = setup.tile([32, CHUNK], F32)
    nc.vector.tensor_tensor_scan(out=cl[:], data0=als[:], data1=als[:],
                                 initial=0.0, op0=OP.add, op1=OP.bypass)
    # exclusive chunk-offset prefix + per-seq final C
    tot = cl[:, CHUNK - 1:CHUNK]
    ps_off = ps_sm.tile([32, 1], F32, tag="sm")
    nc.tensor.matmul(ps_off[:], lt32[:], tot, start=True, stop=True,
                     skip_group_check=True)
    ps_fs = ps_sm.tile([SPC, 1], F32, tag="sm")
    nc.tensor.matmul(ps_fs[:], sum32[:], tot, start=True, stop=True,
                     skip_group_check=True)
    cfin2 = setup.tile([SPC, 1], F32)
    nc.vector.tensor_copy(out=cfin2[:], in_=ps_fs[:])
    ps_cfT = ps_sm.tile([1, SPC], F32, tag="sm")
    nc.tensor.transpose(ps_cfT[:], cfin2[:], id32[0:SPC, 0:SPC])
    cfinT = setup.tile([1, SPC], F32)
    nc.vector.tensor_copy(out=cfinT[:], in_=ps_cfT[:])

    off_sb = setup.tile([32, 1], F32)
    nc.vector.tensor_copy(out=off_sb[:], in_=ps_off[:])
    ps_o = ps_sm.tile([1, 32], F32, tag="sm")
    nc.tensor.transpose(ps_o[:], off_sb[:], id32[:])
    offT0 = setup.tile([1, 32], F32)
    nc.vector.tensor_copy(out=offT0[:], in_=ps_o[:])
    scr_off = nc.dram_tensor("scr_off", [1, 32], F32, kind="Internal").ap()
    nc.sync.dma_start(scr_off[:], offT0[:])
    offh = setup.tile([CHUNK, 32], F32)
    nc.gpsimd.dma_start(offh[:], _bcast_ap(scr_off[0:1, :], CHUNK))

    # transposed local-cumsum columns: CEcur[f,cidx]=Cl[cidx,f], CEprev shifted
    clp = setup.tile([32, CHUNK], F32)
    nc.vector.memset(clp[:, 0:1], 0.0)
    nc.vector.tensor_copy(out=clp[:, 1:CHUNK], in_=cl[:, 0:CHUNK - 1])
    ps_t1 = ps_sm.tile([CHUNK, 32], F32, tag="aux")
    nc.tensor.transpose(ps_t1[:], cl[:], id32[:])
    cecur = setup.tile([CHUNK, 32], F32)
    nc.vector.tensor_copy(out=cecur[:], in_=ps_t1[:])
    ps_t2 = ps_sm.tile([CHUNK, 32], F32, tag="aux")
    nc.tensor.transpose(ps_t2[:], clp[:], id32[:])
    ceprev = setup.tile([CHUNK, 32], F32)
    nc.vector.tensor_copy(out=ceprev[:], in_=ps_t2[:])
    # fold chunk offset into the ACT bias columns
    bias_a = setup.tile([CHUNK, 32], F32)
    nc.vector.tensor_tensor(out=bias_a[:], in0=cecur[:], in1=offh[:], op=OP.add)
    bias_b = setup.tile([CHUNK, 32], F32)
    nc.vector.tensor_tensor(out=bias_b[:], in0=ceprev[:], in1=offh[:], op=OP.add)

    # ---- main loop ----
    for s in range(SPC):
        counts_ps = ps_sm.tile([1, NTOK], F32, tag="aux")
        acc_a0 = ps_acc.tile([128, 512], F32, tag="a0")
        acc_b0 = ps_acc.tile([128, 256], F32, tag="b0")
        acc_a1 = ps_acc.tile([128, 512], F32, tag="a1")
        acc_b1 = ps_acc.tile([128, 256], F32, tag="b1")
        for c in range(NCHUNK):
            cidx = 16 * s + c
            ht = hpool.tile([CHUNK, D], F32, tag="h")
            nc.gpsimd.dma_start(ht[:], h_in[s, c * CHUNK:(c + 1) * CHUNK, :])
            r1b = work.tile([CHUNK, NTOK], F32, tag="r1b")
            nc.scalar.activation(out=r1b[:], in_=negio[:], func=AF.Relu,
                                 bias=bias_b[:, cidx:cidx + 1], scale=1.0)
            r1a = work.tile([CHUNK, NTOK], F32, tag="r1a")
            nc.scalar.activation(out=r1a[:], in_=negio[:], func=AF.Relu,
                                 bias=bias_a[:, cidx:cidx + 1], scale=1.0)
            pb = work.tile([CHUNK, NTOK], F32, tag="pb")
            nc.vector.tensor_scalar(out=pb[:], in0=r1b[:], scalar1=1.0,
                                    scalar2=None, op0=OP.min)
            wt = work.tile([CHUNK, NTOK], F16, tag="w")
            nc.vector.scalar_tensor_tensor(out=wt[:], in0=r1a[:], scalar=1.0,
                                           in1=pb[:], op0=OP.min,
                                           op1=OP.subtract)
            ind = work.tile([CHUNK, NTOK], BF16, tag="ind")
            nc.vector.tensor_scalar(out=ind[:], in0=r1a[:], scalar1=1.0,
                                    scalar2=None, op0=OP.is_lt)
            hrr = work.tile([CHUNK, D], F16, tag="hr")
            nc.vector.tensor_copy(out=hrr[:], in_=ht[:])
            st, sp = (c == 0), (c == NCHUNK - 1)
            nc.tensor.matmul(counts_ps[:], ones_b[:], ind[:], start=st, stop=sp,
                             skip_group_check=True)
            nc.tensor.matmul(acc_a0[:], wt[:, 0:128], hrr[:, 0:512], start=st,
                             stop=sp, skip_group_check=True)
            nc.tensor.matmul(acc_b0[:], wt[:, 0:128], hrr[:, 512:768], start=st,
                             stop=sp, skip_group_check=True)
            nc.tensor.matmul(acc_a1[:], wt[:, 128:256], hrr[:, 0:512], start=st,
                             stop=sp, skip_group_check=True)
            nc.tensor.matmul(acc_b1[:], wt[:, 128:256], hrr[:, 512:768], start=st,
                             stop=sp, skip_group_check=True)

        # ---- finalize sequence s ----
        csb = fin.tile([1, NTOK], F32, tag="csb")
        nc.vector.tensor_copy(out=csb[:], in_=counts_ps[:])
        ltc = fin.tile([1, NTOK], F32, tag="ltc")
        nc.vector.tensor_scalar(out=ltc[:], in0=csb[:], scalar1=float(T),
                                scalar2=None, op0=OP.is_lt)
        ft = fin.tile([1, 1], F32, tag="ft")
        nc.vector.reduce_sum(out=ft[:], in_=ltc[:], axis=mybir.AxisListType.X)
        frac = fin.tile([1, 1], F32, tag="frac")
        nc.vector.tensor_tensor(out=frac[:], in0=cfinT[:, s:s + 1], in1=ft[:],
                                op=OP.subtract)
        tailf = fin.tile([1, 1], F32, tag="tailf")
        nc.vector.tensor_scalar(out=tailf[:], in0=frac[:], scalar1=1e-4,
                                scalar2=None, op0=OP.is_gt)
        ntokf = fin.tile([1, 1], F32, tag="ntokf")
        nc.vector.tensor_tensor(out=ntokf[:], in0=ft[:], in1=tailf[:], op=OP.add)
        tli = fin.tile([1, 1], I32, tag="tli")
        nc.vector.tensor_copy(out=tli[:], in_=ntokf[:])
        nc.sync.dma_start(tl_out[s:s + 1].rearrange("(a b) -> a b", b=1), tli[:])

        valid = fin.tile([1, NTOK], F32, tag="valid")
        nc.vector.tensor_scalar(out=valid[:], in0=kidx[:], scalar1=ntokf[:],
                                scalar2=None, op0=OP.is_lt)
        isfire = fin.tile([1, NTOK], F32, tag="isfire")
        nc.vector.tensor_scalar(out=isfire[:], in0=kidx[:], scalar1=ft[:],
                                scalar2=None, op0=OP.is_lt)
        c1 = fin.tile([1, NTOK], F32, tag="c1")
        nc.vector.tensor_scalar(out=c1[:], in0=csb[:], scalar1=1.0, scalar2=None,
                                op0=OP.add)
        endsA = fin.tile([1, NTOK], F32, tag="endsA")
        nc.vector.tensor_tensor(out=endsA[:], in0=c1[:], in1=isfire[:], op=OP.mult)
        tailm = fin.tile([1, NTOK], F32, tag="tailm")
        nc.vector.tensor_tensor(out=tailm[:], in0=valid[:], in1=isfire[:],
                                op=OP.subtract)
        ends = fin.tile([1, NTOK], F32, tag="ends")
        nc.vector.scalar_tensor_tensor(out=ends[:], in0=tailm[:],
                                       scalar=float(T), in1=endsA[:],
                                       op0=OP.mult, op1=OP.add)
        ssh = fin.tile([1, NTOK], F32, tag="ssh")
        nc.vector.memset(ssh[:, 0:1], 0.0)
        nc.vector.tensor_copy(out=ssh[:, 1:NTOK], in_=csb[:, 0:NTOK - 1])
        starts = fin.tile([1, NTOK], F32, tag="starts")
        nc.vector.tensor_tensor(out=starts[:], in0=ssh[:], in1=valid[:],
                                op=OP.mult)
        ends_i = fin.tile([1, NTOK], I32, tag="ends_i")
        nc.vector.tensor_copy(out=ends_i[:], in_=ends[:])
        starts_i = fin.tile([1, NTOK], I32, tag="starts_i")
        nc.vector.tensor_copy(out=starts_i[:], in_=starts[:])
        nc.sync.dma_start(
            bd_out[s, 0:NTOK, 0].rearrange("(one n) -> one n", one=1), starts_i[:])
        nc.sync.dma_start(
            bd_out[s, 0:NTOK, 1].rearrange("(one n) -> one n", one=1), ends_i[:])

        ct0 = drain.tile([128, D], F32, tag="ct")
        nc.scalar.activation(out=ct0[:, 0:512], in_=acc_a0[:], func=AF.Copy)
        nc.scalar.activation(out=ct0[:, 512:768], in_=acc_b0[:], func=AF.Copy)
        nc.sync.dma_start(c_out[s, 0:128, :], ct0[:])
        ct1 = drain.tile([128, D], F32, tag="ct")
        nc.scalar.activation(out=ct1[:, 0:512], in_=acc_a1[:], func=AF.Copy)
        nc.scalar.activation(out=ct1[:, 512:768], in_=acc_b1[:], func=AF.Copy)
        nc.sync.dma_start(c_out[s, 128:256, :], ct1[:])
        # c rows 256:384 and bounds rows 256:384 stay at the pre-zeroed output.


def build_program():
    nc = bacc.Bacc("TRN2", target_bir_lowering=False, debug=False,
                   enable_asserts=False, num_devices=NCORES)
    h_in = nc.dram_tensor("h_in", [SPC, T, D], F32, kind="ExternalInput").ap()
    a_in = nc.dram_tensor("alpha_in", [SPC, T], F32, kind="ExternalInput").ap()
    tl_in = nc.dram_tensor("tl_in", [SPC], I32, kind="ExternalInput").ap()
    c_out = nc.dram_tensor("c_out", [SPC, NMAX, D], F32, kind="ExternalOutput").ap()
    tl_out = nc.dram_tensor("tl_out", [SPC], I32, kind="ExternalOutput").ap()
    bd_out = nc.dram_tensor("bd_out", [SPC, NMAX, 2], I32,
                            kind="ExternalOutput").ap()
    with tile.TileContext(nc) as tc:
        with ExitStack() as ctx:
            _body(ctx, tc, (c_out, tl_out, bd_out), (h_in, a_in, tl_in))
    nc.compile()
    return nc


_nc_cache = None


def kernel(h, alpha, target_len, **_unused):
    global _nc_cache
    if _nc_cache is None:
        _nc_cache = build_program()
    nc = _nc_cache
    h = np.ascontiguousarray(np.asarray(h, dtype=np.float32))
    alpha = np.ascontiguousarray(np.asarray(alpha, dtype=np.float32))
    target_len = np.ascontiguousarray(np.asarray(target_len, dtype=np.int32))
    in_maps = []
    for i in range(NCORES):
        s0 = i * SPC
        in_maps.append({
            "h_in": np.ascontiguousarray(h[s0:s0 + SPC]),
            "alpha_in": np.ascontiguousarray(alpha[s0:s0 + SPC]),
            "tl_in": np.ascontiguousarray(target_len[s0:s0 + SPC]),
        })
    res = run_bass_kernel_spmd(nc, in_maps, core_ids=list(range(NCORES)))
    c = np.concatenate([r["c_out"] for r in res.results], axis=0)
    tl = np.concatenate([r["tl_out"] for r in res.results], axis=0)
    bd = np.concatenate([r["bd_out"] for r in res.results], axis=0)
    return c.astype(np.float32), tl.astype(np.int32), bd.astype(np.int32)


# revision 12
# speedup vs baseline: 1.5958x; 1.2041x over previous
"""CIF (continuous integrate-and-fire) Trainium2 Bass kernel.

Math: after scaling alpha so it sums to target_len, the CIF scan is a
segment-reduce: token k = sum_t W[k,t] * h[t] with
W[k,t] = |[k,k+1) ∩ [C_{t-1}, C_t)|  (C = cumsum of scaled alpha), i.e.
W[t->row, k->col] = clip01(C_t - k) - clip01(C_{t-1} - k).
Fire frame of token k: t_k = #{t : C_t < k+1}; boundaries[k] = [t_{k-1}, t_k+1],
tail token gets [t_{F-1}, T].  All computed on-device; batch is sharded
2 sequences per core across 8 cores.

Precision: C is computed two-level (per-125-frame scan + chunk-offset prefix),
with chunk totals split into an exact 2^-10-grid part and a tiny residual so
the offset prefix sums are exact; thresholds are evaluated as
(offset_hi - k) + offset_lo + C_local, keeping comparisons at ~1e-5 accuracy.
"""

import numpy as np
from contextlib import ExitStack

import concourse.bacc as bacc
import concourse.bass as bass
import concourse.tile as tile
from concourse import mybir
from concourse.bass_utils import run_bass_kernel_spmd

B, T, D = 16, 2000, 768
NCORES = 8
SPC = 2              # sequences per core
CHUNK = 125
NCHUNK = 16          # 2000 = 16 * 125
NTOK = 256           # max token index is 255 (target_len < 256)
NMAX = 384
MAGIC = 12582912.0   # 1.5 * 2^23: fp32 round-to-int via add/sub

F32 = mybir.dt.float32
F32R = mybir.dt.float32r
F16 = mybir.dt.float16
I32 = mybir.dt.int32
BF16 = mybir.dt.bfloat16
OP = mybir.AluOpType
AF = mybir.ActivationFunctionType


def _consts():
    lt32 = np.zeros((32, 32), np.float32)   # lt32[q,p]=1 iff same 16-block, q%16<p%16
    for q in range(32):
        for p in range(32):
            if q // 16 == p // 16 and q % 16 < p % 16:
                lt32[q, p] = 1.0
    sum32 = np.zeros((32, 2), np.float32)   # sum32[q,s]=1 iff q//16==s
    for q in range(32):
        sum32[q, q // 16] = 1.0
    id32 = np.eye(32, dtype=np.float32)
    b2 = np.zeros((2, 32), np.float32)      # b2[s,p]=1 iff p//16==s
    for p in range(32):
        b2[p // 16, p] = 1.0
    return lt32, sum32, id32, b2


def _bcast_ap(ap, n):
    """Partition-broadcast source AP for DMA: read the same row n times.

    `ap` must be a [1, m] (or [1]-leading) access pattern; the leading
    singleton is replaced by a zero-step dim of count n."""
    return bass.AP(tensor=ap.tensor, offset=ap.offset,
                   ap=[[0, n]] + [list(d) for d in ap.ap[1:]])


def _body(ctx, tc, outs, ins):
    nc = tc.nc
    c_out, tl_out, bd_out = outs
    h_in, a_in, tl_in = ins

    const = ctx.enter_context(tc.tile_pool(name="const", bufs=1))
    setup = ctx.enter_context(tc.tile_pool(name="setup", bufs=1))
    hpool = ctx.enter_context(tc.tile_pool(name="hp", bufs=6))
    work = ctx.enter_context(tc.tile_pool(name="work", bufs=6))
    drain = ctx.enter_context(tc.tile_pool(name="drain", bufs=2))
    fin = ctx.enter_context(tc.tile_pool(name="fin", bufs=2))
    ps_acc = ctx.enter_context(tc.tile_pool(name="psacc", bufs=1, space="PSUM"))
    ps_sm = ctx.enter_context(tc.tile_pool(name="pssm", bufs=2, space="PSUM"))

    # ---- static tiles (constants embedded in the NEFF) ----
    lt32_np, sum32_np, id32_np, b2_np = _consts()
    lt32 = const.tile([32, 32], F32)
    nc.sync.dma_start(lt32[:], nc.inline_tensor(lt32_np, "lt32_c").ap())
    sum32 = const.tile([32, 2], F32)
    nc.sync.dma_start(sum32[:], nc.inline_tensor(sum32_np, "sum32_c").ap())
    id32 = const.tile([32, 32], F32)
    nc.sync.dma_start(id32[:], nc.inline_tensor(id32_np, "id32_c").ap())
    b2t = const.tile([2, 32], F32)
    nc.sync.dma_start(b2t[:], nc.inline_tensor(b2_np, "b2_c").ap())
    ones_row = const.tile([1, CHUNK], F32)
    nc.vector.memset(ones_row[:], 1.0)

    negio_i = const.tile([CHUNK, NTOK], I32)
    nc.gpsimd.iota(negio_i[:], pattern=[[1, NTOK]], base=0, channel_multiplier=0)
    negio = const.tile([CHUNK, NTOK], F32)
    nc.vector.tensor_scalar(out=negio[:], in0=negio_i[:], scalar1=-1.0,
                            scalar2=None, op0=OP.mult)
    kidx_i = const.tile([1, NTOK], I32)
    nc.gpsimd.iota(kidx_i[:], pattern=[[1, NTOK]], base=0, channel_multiplier=0)
    kidx = const.tile([1, NTOK], F32)
    nc.vector.tensor_copy(out=kidx[:], in_=kidx_i[:])
    ones_b = const.tile([CHUNK, 1], BF16)
    nc.vector.memset(ones_b[:], 1.0)

    # ---- per-core setup: scale, scan, offsets ----
    al = setup.tile([32, CHUNK], F32)
    nc.sync.dma_start(al[:], a_in.rearrange("s (c f) -> (s c) f", f=CHUNK))
    tl_i = setup.tile([SPC, 1], I32)
    nc.sync.dma_start(tl_i[:], tl_in.rearrange("(s one) -> s one", one=1))
    tlf = setup.tile([SPC, 1], F32)
    nc.vector.tensor_copy(out=tlf[:], in_=tl_i[:])

    tot_raw = setup.tile([32, 1], F32)
    nc.vector.reduce_sum(out=tot_raw[:], in_=al[:], axis=mybir.AxisListType.X)
    ps_fsr = ps_sm.tile([SPC, 1], F32, tag="sm")
    nc.tensor.matmul(ps_fsr[:], sum32[:], tot_raw[:], start=True, stop=True,
                     skip_group_check=True)
    # scale = tl / (sum + 1e-6), reciprocal + 1 Newton step
    dn = setup.tile([SPC, 1], F32)
    nc.vector.tensor_scalar(out=dn[:], in0=ps_fsr[:], scalar1=1e-6, scalar2=None,
                            op0=OP.add)
    r0 = setup.tile([SPC, 1], F32)
    nc.vector.reciprocal(out=r0[:], in_=dn[:])
    t0 = setup.tile([SPC, 1], F32)
    nc.vector.tensor_tensor(out=t0[:], in0=dn[:], in1=r0[:], op=OP.mult)
    u0 = setup.tile([SPC, 1], F32)
    nc.vector.tensor_scalar(out=u0[:], in0=t0[:], scalar1=-1.0, scalar2=2.0,
                            op0=OP.mult, op1=OP.add)
    r1n = setup.tile([SPC, 1], F32)
    nc.vector.tensor_tensor(out=r1n[:], in0=r0[:], in1=u0[:], op=OP.mult)
    scl = setup.tile([SPC, 1], F32)
    nc.vector.tensor_tensor(out=scl[:], in0=tlf[:], in1=r1n[:], op=OP.mult)
    ps_sbc = ps_sm.tile([32, 1], F32, tag="sm")
    nc.tensor.matmul(ps_sbc[:], b2t[:], scl[:], start=True, stop=True,
                     skip_group_check=True)
    sbc = setup.tile([32, 1], F32)
    nc.vector.tensor_copy(out=sbc[:], in_=ps_sbc[:])

    als = setup.tile([32, CHUNK], F32)
    nc.vector.tensor_scalar(out=als[:], in0=al[:], scalar1=sbc[:], scalar2=None,
                            op0=OP.mult)
    cl = setup.tile([32, CHUNK], F32)
    nc.vector.tensor_tensor_scan(out=cl[:], data0=als[:], data1=als[:],
                                 initial=0.0, op0=OP.add, op1=OP.bypass)
    # exclusive chunk-offset prefix + per-seq final C
    tot = cl[:, CHUNK - 1:CHUNK]
    ps_off = ps_sm.tile([32, 1], F32, tag="sm")
    nc.tensor.matmul(ps_off[:], lt32[:], tot, start=True, stop=True,
                     skip_group_check=True)
    ps_fs = ps_sm.tile([SPC, 1], F32, tag="sm")
    nc.tensor.matmul(ps_fs[:], sum32[:], tot, start=True, stop=True,
                     skip_group_check=True)
    cfin2 = setup.tile([SPC, 1], F32)
    nc.vector.tensor_copy(out=cfin2[:], in_=ps_fs[:])
    ps_cfT = ps_sm.tile([1, SPC], F32, tag="sm")
    nc.tensor.transpose(ps_cfT[:], cfin2[:], id32[0:SPC, 0:SPC])
    cfinT = setup.tile([1, SPC], F32)
    nc.vector.tensor_copy(out=cfinT[:], in_=ps_cfT[:])

    off_sb = setup.tile([32, 1], F32)
    nc.vector.tensor_copy(out=off_sb[:], in_=ps_off[:])
    ps_o = ps_sm.tile([1, 32], F32, tag="sm")
    nc.tensor.transpose(ps_o[:], off_sb[:], id32[:])
    offT0 = setup.tile([1, 32], F32)
    nc.vector.tensor_copy(out=offT0[:], in_=ps_o[:])

    # transposed local-cumsum columns: CEcur[f,cidx]=Cl[cidx,f], CEprev shifted
    clp = setup.tile([32, CHUNK], F32)
    nc.vector.memset(clp[:, 0:1], 0.0)
    nc.vector.tensor_copy(out=clp[:, 1:CHUNK], in_=cl[:, 0:CHUNK - 1])
    ps_t1 = ps_sm.tile([CHUNK, 32], F32, tag="aux")
    nc.tensor.matmul(ps_t1[:], cl[:], id32[:], is_transpose=True, start=True,
                     stop=False, skip_group_check=True)
    nc.tensor.matmul(ps_t1[:], ones_row[:], offT0[:], start=False, stop=True,
                     skip_group_check=True)
    bias_a = setup.tile([CHUNK, 32], F32)
    nc.vector.tensor_copy(out=bias_a[:], in_=ps_t1[:])
    ps_t2 = ps_sm.tile([CHUNK, 32], F32, tag="aux")
    nc.tensor.matmul(ps_t2[:], clp[:], id32[:], is_transpose=True, start=True,
                     stop=False, skip_group_check=True)
    nc.tensor.matmul(ps_t2[:], ones_row[:], offT0[:], start=False, stop=True,
                     skip_group_check=True)
    bias_b = setup.tile([CHUNK, 32], F32)
    nc.vector.tensor_copy(out=bias_b[:], in_=ps_t2[:])

    # ---- main loop ----
    for s in range(SPC):
        counts_ps = ps_sm.tile([1, NTOK], F32, tag="aux")
        acc_a0 = ps_acc.tile([128, 512], F32, tag="a0")
        acc_b0 = ps_acc.tile([128, 256], F32, tag="b0")
        acc_a1 = ps_acc.tile([128, 512], F32, tag="a1")
        acc_b1 = ps_acc.tile([128, 256], F32, tag="b1")
        for c in range(NCHUNK):
            cidx = 16 * s + c
            ht = hpool.tile([CHUNK, D], F32, tag="h")
            nc.gpsimd.dma_start(ht[:], h_in[s, c * CHUNK:(c + 1) * CHUNK, :])
            r1b = work.tile([CHUNK, NTOK], F32, tag="r1b")
            nc.scalar.activation(out=r1b[:], in_=negio[:], func=AF.Relu,
                                 bias=bias_b[:, cidx:cidx + 1], scale=1.0)
            r1a = work.tile([CHUNK, NTOK], F32, tag="r1a")
            nc.scalar.activation(out=r1a[:], in_=negio[:], func=AF.Relu,
                                 bias=bias_a[:, cidx:cidx + 1], scale=1.0)
            pb = work.tile([CHUNK, NTOK], F32, tag="pb")
            nc.vector.tensor_scalar(out=pb[:], in0=r1b[:], scalar1=1.0,
                                    scalar2=None, op0=OP.min)
            wt = work.tile([CHUNK, NTOK], F16, tag="w")
            nc.vector.scalar_tensor_tensor(out=wt[:], in0=r1a[:], scalar=1.0,
                                           in1=pb[:], op0=OP.min,
                                           op1=OP.subtract)
            ind = work.tile([CHUNK, NTOK], BF16, tag="ind")
            nc.vector.tensor_scalar(out=ind[:], in0=r1a[:], scalar1=1.0,
                                    scalar2=None, op0=OP.is_lt)
            hrr = work.tile([CHUNK, D], F16, tag="hr")
            nc.vector.tensor_copy(out=hrr[:], in_=ht[:])
            st, sp = (c == 0), (c == NCHUNK - 1)
            nc.tensor.matmul(counts_ps[:], ones_b[:], ind[:], start=st, stop=sp,
                             skip_group_check=True)
            nc.tensor.matmul(acc_a0[:], wt[:, 0:128], hrr[:, 0:512], start=st,
                             stop=sp, skip_group_check=True)
            nc.tensor.matmul(acc_b0[:], wt[:, 0:128], hrr[:, 512:768], start=st,
                             stop=sp, skip_group_check=True)
            nc.tensor.matmul(acc_a1[:], wt[:, 128:256], hrr[:, 0:512], start=st,
                             stop=sp, skip_group_check=True)
            nc.tensor.matmul(acc_b1[:], wt[:, 128:256], hrr[:, 512:768], start=st,
                             stop=sp, skip_group_check=True)

        # ---- finalize sequence s ----
        csb = fin.tile([1, NTOK], F32, tag="csb")
        nc.vector.tensor_copy(out=csb[:], in_=counts_ps[:])
        ltc = fin.tile([1, NTOK], F32, tag="ltc")
        nc.vector.tensor_scalar(out=ltc[:], in0=csb[:], scalar1=float(T),
                                scalar2=None, op0=OP.is_lt)
        ft = fin.tile([1, 1], F32, tag="ft")
        nc.vector.reduce_sum(out=ft[:], in_=ltc[:], axis=mybir.AxisListType.X)
        frac = fin.tile([1, 1], F32, tag="frac")
        nc.vector.tensor_tensor(out=frac[:], in0=cfinT[:, s:s + 1], in1=ft[:],
                                op=OP.subtract)
        tailf = fin.tile([1, 1], F32, tag="tailf")
        nc.vector.tensor_scalar(out=tailf[:], in0=frac[:], scalar1=1e-4,
                                scalar2=None, op0=OP.is_gt)
        ntokf = fin.tile([1, 1], F32, tag="ntokf")
        nc.vector.tensor_tensor(out=ntokf[:], in0=ft[:], in1=tailf[:], op=OP.add)
        tli = fin.tile([1, 1], I32, tag="tli")
        nc.vector.tensor_copy(out=tli[:], in_=ntokf[:])
        nc.sync.dma_start(tl_out[s:s + 1].rearrange("(a b) -> a b", b=1), tli[:])

        valid = fin.tile([1, NTOK], F32, tag="valid")
        nc.vector.tensor_scalar(out=valid[:], in0=kidx[:], scalar1=ntokf[:],
                                scalar2=None, op0=OP.is_lt)
        isfire = fin.tile([1, NTOK], F32, tag="isfire")
        nc.vector.tensor_scalar(out=isfire[:], in0=kidx[:], scalar1=ft[:],
                                scalar2=None, op0=OP.is_lt)
        c1 = fin.tile([1, NTOK], F32, tag="c1")
        nc.vector.tensor_scalar(out=c1[:], in0=csb[:], scalar1=1.0, scalar2=None,
                                op0=OP.add)
        endsA = fin.tile([1, NTOK], F32, tag="endsA")
        nc.vector.tensor_tensor(out=endsA[:], in0=c1[:], in1=isfire[:], op=OP.mult)
        tailm = fin.tile([1, NTOK], F32, tag="tailm")
        nc.vector.tensor_tensor(out=tailm[:], in0=valid[:], in1=isfire[:],
                                op=OP.subtract)
        ends = fin.tile([1, NTOK], F32, tag="ends")
        nc.vector.scalar_tensor_tensor(out=ends[:], in0=tailm[:],
                                       scalar=float(T), in1=endsA[:],
                                       op0=OP.mult, op1=OP.add)
        ssh = fin.tile([1, NTOK], F32, tag="ssh")
        nc.vector.memset(ssh[:, 0:1], 0.0)
        nc.vector.tensor_copy(out=ssh[:, 1:NTOK], in_=csb[:, 0:NTOK - 1])
        starts = fin.tile([1, NTOK], F32, tag="starts")
        nc.vector.tensor_tensor(out=starts[:], in0=ssh[:], in1=valid[:],
                                op=OP.mult)
        ends_i = fin.tile([1, NTOK], I32, tag="ends_i")
        nc.vector.tensor_copy(out=ends_i[:], in_=ends[:])
        starts_i = fin.tile([1, NTOK], I32, tag="starts_i")
        nc.vector.tensor_copy(out=starts_i[:], in_=starts[:])
        nc.sync.dma_start(
            bd_out[s, 0:NTOK, 0].rearrange("(one n) -> one n", one=1), starts_i[:])
        nc.sync.dma_start(
            bd_out[s, 0:NTOK, 1].rearrange("(one n) -> one n", one=1), ends_i[:])

        ct0 = drain.tile([128, D], F32, tag="ct")
        nc.scalar.activation(out=ct0[:, 0:512], in_=acc_a0[:], func=AF.Copy)
        nc.scalar.activation(out=ct0[:, 512:768], in_=acc_b0[:], func=AF.Copy)
        nc.sync.dma_start(c_out[s, 0:128, :], ct0[:])
        ct1 = drain.tile([128, D], F32, tag="ct")
        nc.scalar.activation(out=ct1[:, 0:512], in_=acc_a1[:], func=AF.Copy)
        nc.scalar.activation(out=ct1[:, 512:768], in_=acc_b1[:], func=AF.Copy)
        nc.sync.dma_start(c_out[s, 128:256, :], ct1[:])
        # c rows 256:384 and bounds rows 256:384 stay at the pre-zeroed output.


def build_program():
    nc = bacc.Bacc("TRN2", target_bir_lowering=False, debug=False,
                   enable_asserts=False, num_devices=NCORES)
    h_in = nc.dram_tensor("h_in", [SPC, T, D], F32, kind="ExternalInput").ap()
    a_in = nc.dram_tensor("alpha_in", [SPC, T], F32, kind="ExternalInput").ap()
    tl_in = nc.dram_tensor("tl_in", [SPC], I32, kind="ExternalInput").ap()
    c_out = nc.dram_tensor("c_out", [SPC, NMAX, D], F32, kind="ExternalOutput").ap()
    tl_out = nc.dram_tensor("tl_out", [SPC], I32, kind="ExternalOutput").ap()
    bd_out = nc.dram_tensor("bd_out", [SPC, NMAX, 2], I32,
                            kind="ExternalOutput").ap()
    with tile.TileContext(nc) as tc:
        with ExitStack() as ctx:
            _body(ctx, tc, (c_out, tl_out, bd_out), (h_in, a_in, tl_in))
    nc.compile()
    return nc


_nc_cache = None


def kernel(h, alpha, target_len, **_unused):
    global _nc_cache
    if _nc_cache is None:
        _nc_cache = build_program()
    nc = _nc_cache
    h = np.ascontiguousarray(np.asarray(h, dtype=np.float32))
    alpha = np.ascontiguousarray(np.asarray(alpha, dtype=np.float32))
    target_len = np.ascontiguousarray(np.asarray(target_len, dtype=np.int32))
    in_maps = []
    for i in range(NCORES):
        s0 = i * SPC
        in_maps.append({
            "h_in": np.ascontiguousarray(h[s0:s0 + SPC]),
            "alpha_in": np.ascontiguousarray(alpha[s0:s0 + SPC]),
            "tl_in": np.ascontiguousarray(target_len[s0:s0 + SPC]),
        })
    res = run_bass_kernel_spmd(nc, in_maps, core_ids=list(range(NCORES)))
    c = np.concatenate([r["c_out"] for r in res.results], axis=0)
    tl = np.concatenate([r["tl_out"] for r in res.results], axis=0)
    bd = np.concatenate([r["bd_out"] for r in res.results], axis=0)
    return c.astype(np.float32), tl.astype(np.int32), bd.astype(np.int32)
